# revision 1
# baseline (speedup 1.0000x reference)
"""GPT forward pass on 8 Trainium2 NeuronCores (Bass/Tile).

Model: B=2, S=1024, D=1024, H=16 heads (hd=64), L=6 layers, V=50257,
tied embedding head.

Sharding: DP2 x TP4. Cores 0-3 compute batch element 0, cores 4-7
batch element 1. Within each group of 4: attention is head-sharded
(4 heads/core), the FFN hidden dim is column/row sharded (1024/core),
and the tied logit matrix is vocab-sharded (12565 rows/core, padded
to 12800). Two AllReduces per layer (post-attention, post-FFN) over
each group of 4, token-chunked (2 x 512) so collectives overlap
compute.

On-device layout: activations are feature-major ("transposed"):
x[p, k, t] = X^T[128k + p, t]. All matmuls take weights [in, out] as
the stationary operand and activations [in, tokens] as the moving
operand, producing outputs feature-major with zero activation
transposes. LayerNorm gains/biases and the attention 1/sqrt(hd) scale
are folded into the adjacent weights on the host, so the device only
computes the pure normalization (x - mean) * rsqrt(var + eps), with
stats via ones-matmuls on the PE and per-token broadcasts via GpSimd.

Matmuls run in float32r (full PE rate, ~tf32 precision).
"""

import sys

sys.path.insert(0, "/opt/trn_rl_repo")

import contextlib

import numpy as np

import concourse.bacc as bacc
import concourse.mybir as mybir
import concourse.tile as tile
from concourse.bass import ts
from concourse.bass_utils import run_bass_kernel_spmd

F32 = mybir.dt.float32
F32R = mybir.dt.float32r
BF16 = mybir.dt.bfloat16
AF = mybir.ActivationFunctionType
ALU = mybir.AluOpType

# Model dims
B, S, D, H, L, V = 2, 1024, 1024, 16, 6, 50257
HD = D // H           # 64
DFF = 4 * D           # 4096
N_CORES = 8
TP = 4                # tensor-parallel group size
HPC = H // TP         # heads per core = 4
DQ = HPC * HD         # per-core qkv width = 256
FFC = DFF // TP       # per-core ffn hidden = 1024
KD = D // 128         # 8 feature tiles
T = S                 # tokens per core (one batch element)
TCH = 512             # token chunk for AR pipelining
NTC = T // TCH        # 2
VS = 12565            # vocab rows per core (last core: 12562)
VSP = 12800           # padded vocab rows per core
VB = 256              # vocab tile for the logit matmul
MASK_VAL = -60.0

REPLICA_GROUPS = [[0, 1, 2, 3], [4, 5, 6, 7]]


def _f(name, l=None):
    return name if l is None else f"{name}{l}"


def build_program(debug_taps=False):
    """Build the SPMD bass program (same instruction stream on all cores)."""
    nc = bacc.Bacc("TRN2", target_bir_lowering=False, debug=False,
                   enable_asserts=True, num_devices=N_CORES)

    inp = {}

    def dram_in(name, shape, dtype=F32R):
        inp[name] = nc.dram_tensor(name, shape, dtype, kind="ExternalInput").ap()
        return inp[name]

    dram_in("x0", [128, KD, T])
    dram_in("ones", [128, 1])
    dram_in("ident", [128, 128])
    dram_in("ident2", [128, 64])
    dram_in("tri", [128, 128])
    for l in range(L):
        dram_in(_f("wq", l), [128, KD, DQ])
        dram_in(_f("wk", l), [128, KD, DQ])
        dram_in(_f("wv", l), [128, KD, DQ])
        dram_in(_f("wo", l), [128, DQ // 128, D])
        dram_in(_f("w1", l), [128, KD, FFC])
        dram_in(_f("w2", l), [128, FFC // 128, D])
        dram_in(_f("bqkv", l), [128, 6], F32)
        dram_in(_f("b1", l), [128, FFC // 128], F32)
        dram_in(_f("b2", l), [128, KD], F32)
    dram_in("emb", [128, KD, VSP])
    logits = nc.dram_tensor("logits", [VSP, T], F32, kind="ExternalOutput").ap()

    taps = {}
    if debug_taps:
        for name in ["n1_0", "q_0", "k_0", "o_0", "xa_0", "x_1", "nf"]:
            shape = {"q_0": [128, DQ // 128, T], "k_0": [128, DQ // 128, T],
                     "o_0": [128, DQ // 128, T]}.get(name, [128, KD, T])
            taps[name] = nc.dram_tensor("dbg_" + name, shape, F32,
                                        kind="ExternalOutput").ap()

    with tile.TileContext(nc) as tc:
        _body(tc, inp, logits, taps)
    nc.compile()
    return nc


def _body(tc, inp, logits, taps):
    nc = tc.nc
    ctx = contextlib.ExitStack()
    with ctx:
        # --- SBUF pools (sizes are KB/partition) ---
        singles = ctx.enter_context(tc.tile_pool(name="singles", bufs=1))
        xp = ctx.enter_context(tc.tile_pool(name="xp", bufs=1))        # 32
        npool = ctx.enter_context(tc.tile_pool(name="npool", bufs=1))  # 32
        tmp = ctx.enter_context(tc.tile_pool(name="tmp", bufs=2))      # 6
        qkv = ctx.enter_context(tc.tile_pool(name="qkv", bufs=1))      # 24
        vh = ctx.enter_context(tc.tile_pool(name="vh", bufs=4))        # 4
        ep = ctx.enter_context(tc.tile_pool(name="ep", bufs=2))        # 8
        et = ctx.enter_context(tc.tile_pool(name="et", bufs=1))        # 16
        op = ctx.enter_context(tc.tile_pool(name="op", bufs=1))        # 8
        hp = ctx.enter_context(tc.tile_pool(name="hp", bufs=1))        # 16
        arr = ctx.enter_context(tc.tile_pool(name="arr", bufs=1))      # 16
        wts = ctx.enter_context(tc.tile_pool(name="wts", bufs=2))      # 12
        embp = ctx.enter_context(tc.tile_pool(name="embp", bufs=2))    # 16
        stat = ctx.enter_context(tc.tile_pool(name="stat", bufs=2))
        bcp = ctx.enter_context(tc.tile_pool(name="bcp", bufs=1))      # 8
        lout = ctx.enter_context(tc.tile_pool(name="lout", bufs=1))
        bias = ctx.enter_context(tc.tile_pool(name="bias", bufs=3))
        stg = ctx.enter_context(tc.tile_pool(name="stg", bufs=2))     # 6
        # --- PSUM pools (8 banks total) ---
        ps_mm = ctx.enter_context(tc.tile_pool(name="ps_mm", bufs=2, space="PSUM"))
        ps_st = ctx.enter_context(tc.tile_pool(name="ps_st", bufs=1, space="PSUM"))
        ps_sc = ctx.enter_context(tc.tile_pool(name="ps_sc", bufs=2, space="PSUM"))
        ps_tr = ctx.enter_context(tc.tile_pool(name="ps_tr", bufs=2, space="PSUM"))
        ps_o = ctx.enter_context(tc.tile_pool(name="ps_o", bufs=1, space="PSUM"))
        # --- DRAM (collective bounce) ---
        dram = ctx.enter_context(tc.tile_pool(name="dram", bufs=4, space="DRAM"))

        # --- constants / persistent ---
        ones_t = singles.tile([128, 1], F32R)
        nc.sync.dma_start(out=ones_t[:], in_=inp["ones"][:])
        ident_t = singles.tile([128, 128], F32R)
        nc.sync.dma_start(out=ident_t[:], in_=inp["ident"][:])
        ident2_t = singles.tile([128, 64], F32R)
        nc.sync.dma_start(out=ident2_t[:], in_=inp["ident2"][:])
        tri_t = singles.tile([128, 128], F32R)
        nc.sync.dma_start(out=tri_t[:], in_=inp["tri"][:])
        eps_t = singles.tile([1, 1], F32)
        nc.vector.memset(eps_t[:], 1e-5)

        xt = xp.tile([128, KD, T], F32R, tag="x")
        nc.sync.dma_start(out=xt[:], in_=inp["x0"][:])

        def layer_norm_chunk(src, dst, c):
            """dst[:,:,c] = (src - mean) * rsqrt(var + eps), feature-major."""
            if True:
                cs = ts(c, TCH)
                s1 = ps_st.tile([1, TCH], F32, tag="st")
                s2 = ps_st.tile([1, TCH], F32, tag="st")
                for k in range(KD):
                    nc.tensor.matmul(s1[:], ones_t[:], src[:, k, cs],
                                     start=(k == 0), stop=(k == KD - 1))
                for k in range(KD):
                    sq = tmp.tile([128, TCH], F32R, tag="tmp")
                    nc.vector.tensor_tensor(
                        out=sq[:], in0=src[:, k, cs].bitcast(F32),
                        in1=src[:, k, cs].bitcast(F32), op=ALU.mult)
                    nc.tensor.matmul(s2[:], ones_t[:], sq[:],
                                     start=(k == 0), stop=(k == KD - 1))
                # finishing math on [1, TCH] rows
                m = stat.tile([1, TCH], F32, tag="sa")
                nc.vector.tensor_scalar_mul(m[:], s1[:], 1.0 / D)
                msq = stat.tile([1, TCH], F32, tag="sb")
                nc.vector.tensor_tensor(out=msq[:], in0=m[:], in1=m[:],
                                        op=ALU.mult)
                var = stat.tile([1, TCH], F32, tag="sb")
                nc.vector.scalar_tensor_tensor(
                    out=var[:], in0=s2[:], scalar=1.0 / D, in1=msq[:],
                    op0=ALU.mult, op1=ALU.subtract)
                rs = stat.tile([1, TCH], F32, tag="sb")
                nc.scalar.activation(rs[:], var[:], AF.Sqrt, bias=eps_t[:])
                nc.vector.reciprocal(rs[:], rs[:])
                a = stat.tile([1, TCH], F32, tag="sa")
                nc.vector.scalar_tensor_tensor(
                    out=a[:], in0=m[:], scalar=-1.0, in1=rs[:],
                    op0=ALU.mult, op1=ALU.mult)
                rB = bcp.tile([128, TCH], F32, tag="rB")
                nc.gpsimd.partition_broadcast(rB[:], rs[:])
                aB = bcp.tile([128, TCH], F32, tag="aB")
                nc.gpsimd.partition_broadcast(aB[:], a[:])
                # dst = src * rB + aB   (two DVE passes per k)
                for k in range(KD):
                    t2 = tmp.tile([128, TCH], F32, tag="tmp")
                    nc.vector.tensor_tensor(
                        out=t2[:], in0=src[:, k, cs].bitcast(F32),
                        in1=rB[:], op=ALU.mult)
                    nc.vector.tensor_tensor(
                        out=dst[:, k, cs], in0=t2[:], in1=aB[:], op=ALU.add)

        def proj_chunk(dst, wname, n_src, mchunks, bias_t, bcol0, c):
            """dst[:, m, c-chunk] (f32r) = W^T @ n_src[c-chunk] + bias."""
            cs = ts(c, TCH)
            for m in range(mchunks):
                wstrip = wts.tile([128, KD, 128], F32R, tag="w")
                nc.sync.dma_start(out=wstrip[:],
                                  in_=inp[wname][:, :, ts(m, 128)])
                pt = ps_mm.tile([128, TCH], F32, tag="mm")
                for k in range(KD):
                    nc.tensor.matmul(pt[:], wstrip[:, k, :],
                                     n_src[:, k, cs],
                                     start=(k == 0), stop=(k == KD - 1))
                nc.scalar.activation(
                    dst[:, m, cs], pt[:], AF.Identity,
                    bias=bias_t[:, bcol0 + m:bcol0 + m + 1])

        def qkv_head_chunk(l, c, n_src, qT, kT, vT, bqkv_t, vhs):
            """QKV projections for token chunk c, plus the v-transposes
            whose key blocks live in chunk c."""
            proj_chunk(qT, _f("wq", l), n_src, DQ // 128, bqkv_t, 0, c)
            proj_chunk(kT, _f("wk", l), n_src, DQ // 128, bqkv_t, 2, c)
            proj_chunk(vT, _f("wv", l), n_src, DQ // 128, bqkv_t, 4, c)
            for h in range(HPC):
                pp = 64 * (h % 2)
                mh = h // 2
                for j in range(4 * c, 4 * c + 4):
                    tp_ = ps_tr.tile([128, 128], F32R, tag="tr")
                    nc.tensor.matmul(tp_[:, 0:HD],
                                     vT[pp:pp + 64, mh, ts(j, 128)],
                                     ident2_t[pp:pp + 64, :],
                                     is_transpose=True,
                                     start=True, stop=True)
                    nc.scalar.copy(vhs[h][:, j, :], tp_[:, 0:HD].bitcast(F32))

        def attn_chunk(l, c, qT, kT, vhs, oT, inject=None, inject_at=1):
            for h in range(HPC):
                if h == inject_at and inject is not None:
                    inject()
                pp = 64 * (h % 2)
                mh = h // 2
                etile = et.tile([128, KD, TCH], F32R, tag="et")
                nkj = 4 * (c + 1)
                for qi in range(4 * c, 4 * c + 4):
                    qs = ts(qi, 128)
                    nkeys = 128 * (qi + 1)
                    erow = ep.tile([128, T], F32R, tag="e")
                    rsum = stat.tile([128, 1], F32, tag="rsum")
                    nchunks = (nkeys + 511) // 512
                    for sc in range(nchunks):
                        w = min(512, nkeys - 512 * sc)
                        last = sc == nchunks - 1
                        spt = ps_sc.tile([128, 512], F32, tag="sc")
                        nc.tensor.matmul(
                            spt[:, :w], qT[pp:pp + 64, mh, qs],
                            kT[pp:pp + 64, mh, 512 * sc:512 * sc + w],
                            start=True, stop=not last)
                        if last:
                            # add the causal mask for the diagonal block
                            nc.tensor.matmul(spt[:, w - 128:w], ident_t[:],
                                             tri_t[:], start=False, stop=True)
                        if sc == 0:
                            nc.scalar.activation(
                                erow[:, :w], spt[:, :w], AF.Exp,
                                accum_out=rsum[:])
                        else:
                            rpart = stat.tile([128, 1], F32, tag="rp")
                            nc.scalar.activation(
                                erow[:, 512 * sc:512 * sc + w],
                                spt[:, :w], AF.Exp, accum_out=rpart[:])
                            nc.vector.tensor_tensor(
                                out=rsum[:], in0=rsum[:], in1=rpart[:],
                                op=ALU.add)
                    nc.vector.reciprocal(rsum[:], rsum[:])
                    en = ep.tile([128, T], F32R, tag="e")
                    nc.scalar.activation(en[:, :nkeys],
                                         erow[:, :nkeys].bitcast(F32),
                                         AF.Copy, scale=rsum[:])
                    for kj in range(qi + 1):
                        tp_ = ps_tr.tile([128, 128], F32R, tag="tr")
                        nc.tensor.matmul(tp_[:], en[:, ts(kj, 128)],
                                         ident_t[:], is_transpose=True,
                                         start=True, stop=True)
                        qo = 128 * (qi - 4 * c)
                        nc.scalar.copy(etile[:, kj, qo:qo + 128],
                                       tp_[:].bitcast(F32))
                po = ps_o.tile([64, TCH], F32, tag="o")
                for kj in range(nkj):
                    lo = max(0, 128 * kj - TCH * c)
                    nc.tensor.matmul(po[:, lo:TCH], vhs[h][:, kj, :],
                                     etile[:, kj, lo:TCH],
                                     start=(kj == 0), stop=(kj == nkj - 1))
                nc.scalar.copy(oT[pp:pp + 64, mh, ts(c, TCH)], po[:])

        def mm_ar_chunk(wname, kchunks, src_tile, c, src_is_chunk):
            """out-partial = W^T @ src for chunk c -> bf16 AllReduce."""
            ar_in = dram.tile([128, KD, TCH], F32, tag="dr")
            ar_out = dram.tile([128, KD, TCH], F32, tag="dr")
            for m in range(KD):
                wstrip = wts.tile([128, KD, 128], F32R, tag="w")
                nc.sync.dma_start(out=wstrip[:, 0:kchunks, :],
                                  in_=inp[wname][:, :, ts(m, 128)])
                pt = ps_mm.tile([128, TCH], F32, tag="mm")
                for k in range(kchunks):
                    s = (src_tile[:, k, :] if src_is_chunk
                         else src_tile[:, k, ts(c, TCH)])
                    nc.tensor.matmul(pt[:], wstrip[:, k, :], s,
                                     start=(k == 0), stop=(k == kchunks - 1))
                st_ = stg.tile([128, TCH], F32, tag="stg")
                nc.scalar.copy(st_[:], pt[:])
                nc.sync.dma_start(out=ar_in[:, m, :], in_=st_[:])
            nc.gpsimd.collective_compute(
                "AllReduce", ALU.add, replica_groups=REPLICA_GROUPS,
                ins=[ar_in.opt()], outs=[ar_out.opt()])
            return ar_out

        # ---------------- prologue: LN1 + QKV of layer 0 ----------------
        n_cur = npool.tile([128, KD, T], F32R, tag="n")
        bqkv_t = bias.tile([128, 6], F32, tag="bias")
        nc.sync.dma_start(out=bqkv_t[:], in_=inp[_f("bqkv", 0)][:])
        qT = qkv.tile([128, DQ // 128, T], F32R, tag="qT")
        kT = qkv.tile([128, DQ // 128, T], F32R, tag="kT")
        vT = qkv.tile([128, DQ // 128, T], F32R, tag="vT")
        vhs = [vh.tile([128, KD, HD], F32R, tag="vh", name=f"vh{i}")
               for i in range(HPC)]
        for c in range(NTC):
            layer_norm_chunk(xt, n_cur, c)
            qkv_head_chunk(0, c, n_cur, qT, kT, vT, bqkv_t, vhs)

        def ffn_w1(l, c, n2, b1_t):
            cs = ts(c, TCH)
            hT = hp.tile([128, FFC // 128, TCH], F32R, tag="h",
                         name=f"hT{l}_{c}")
            for m in range(FFC // 128):
                wstrip = wts.tile([128, KD, 128], F32R, tag="w",
                                  name=f"w1s{l}_{c}_{m}")
                nc.sync.dma_start(out=wstrip[:],
                                  in_=inp[_f("w1", l)][:, :, ts(m, 128)])
                pt = ps_mm.tile([128, TCH], F32, tag="mm", name=f"p1_{l}_{c}_{m}")
                for k in range(KD):
                    nc.tensor.matmul(pt[:], wstrip[:, k, :], n2[:, k, cs],
                                     start=(k == 0), stop=(k == KD - 1))
                nc.scalar.activation(hT[:, m, :], pt[:], AF.Gelu,
                                     bias=b1_t[:, m:m + 1])
            return hT

        def add_f_lnnext(c, ar_f, b2_t, n_next):
            cs = ts(c, TCH)
            art = arr.tile([128, KD, TCH], F32, tag="arr", name=f"artf{c}")
            nc.sync.dma_start(out=art[:], in_=ar_f[c][:])
            for m in range(KD):
                nc.vector.scalar_tensor_tensor(
                    out=xt[:, m, cs], in0=art[:, m, :],
                    scalar=b2_t[:, m:m + 1],
                    in1=xt[:, m, cs].bitcast(F32),
                    op0=ALU.add, op1=ALU.add)
            layer_norm_chunk(xt, n_next, c)

        for l in range(L):
            if "n1_0" in taps and l == 0:
                nc.sync.dma_start(out=taps["n1_0"][:], in_=n_cur[:].bitcast(F32))
            if "q_0" in taps and l == 0:
                nc.sync.dma_start(out=taps["q_0"][:], in_=qT[:].bitcast(F32))
                nc.sync.dma_start(out=taps["k_0"][:], in_=kT[:].bitcast(F32))

            # ---------------- attention + Wo + AR, chunk-pipelined ----------------
            oT = op.tile([128, DQ // 128, T], F32R, tag="oT")
            n2 = npool.tile([128, KD, T], F32R, tag="n")
            ar_a = []

            def add_a(c):
                art = arr.tile([128, KD, TCH], F32, tag="arr",
                               name=f"arta{c}_{l}")
                nc.sync.dma_start(out=art[:], in_=ar_a[c][:])
                nc.vector.tensor_tensor(
                    out=xt[:, :, ts(c, TCH)],
                    in0=xt[:, :, ts(c, TCH)].bitcast(F32),
                    in1=art[:], op=ALU.add)

            def add_a_ln2_c0():
                add_a(0)
                layer_norm_chunk(xt, n2, 0)

            attn_chunk(l, 0, qT, kT, vhs, oT)
            ar_a.append(mm_ar_chunk(_f("wo", l), DQ // 128, oT, 0, False))
            attn_chunk(l, 1, qT, kT, vhs, oT, inject=add_a_ln2_c0)
            ar_a.append(mm_ar_chunk(_f("wo", l), DQ // 128, oT, 1, False))
            if "o_0" in taps and l == 0:
                nc.sync.dma_start(out=taps["o_0"][:], in_=oT[:].bitcast(F32))

            # ------- residual c1 + LN2(c1) overlap FFN(c0) -------
            b1_t = bias.tile([128, FFC // 128], F32, tag="bias")
            nc.sync.dma_start(out=b1_t[:], in_=inp[_f("b1", l)][:])
            b2_t = bias.tile([128, KD], F32, tag="bias")
            nc.sync.dma_start(out=b2_t[:], in_=inp[_f("b2", l)][:])
            add_a(1)
            layer_norm_chunk(xt, n2, 1)
            if "xa_0" in taps and l == 0:
                nc.sync.dma_start(out=taps["xa_0"][:], in_=xt[:].bitcast(F32))

            ar_f = []
            hT0 = ffn_w1(l, 0, n2, b1_t)
            ar_f.append(mm_ar_chunk(_f("w2", l), FFC // 128, hT0, 0, True))
            hT1 = ffn_w1(l, 1, n2, b1_t)

            n_next = npool.tile([128, KD, T], F32R, tag="n")
            ar_f.append(mm_ar_chunk(_f("w2", l), FFC // 128, hT1, 1, True))
            add_f_lnnext(0, ar_f, b2_t, n_next)

            if l < L - 1:
                bqkv_t = bias.tile([128, 6], F32, tag="bias")
                nc.sync.dma_start(out=bqkv_t[:], in_=inp[_f("bqkv", l + 1)][:])
                qT = qkv.tile([128, DQ // 128, T], F32R, tag="qT")
                kT = qkv.tile([128, DQ // 128, T], F32R, tag="kT")
                vT = qkv.tile([128, DQ // 128, T], F32R, tag="vT")
                vhs = [vh.tile([128, KD, HD], F32R, tag="vh",
                               name=f"vh{l + 1}_{i}") for i in range(HPC)]
                qkv_head_chunk(l + 1, 0, n_next, qT, kT, vT, bqkv_t, vhs)
            add_f_lnnext(1, ar_f, b2_t, n_next)
            if l < L - 1:
                qkv_head_chunk(l + 1, 1, n_next, qT, kT, vT, bqkv_t, vhs)
            if "x_1" in taps and l == 0:
                nc.sync.dma_start(out=taps["x_1"][:], in_=xt[:].bitcast(F32))
            n_cur = n_next

        # after the loop, n_cur holds the final layer norm
        nf = n_cur
        if "nf" in taps:
            nc.sync.dma_start(out=taps["nf"][:], in_=nf[:].bitcast(F32))

        # logits^T [VSP, T]: emb strip is the stationary operand, nf moves.
        # A t2=0-only prefix runs first: it depends only on chunk 0 of the
        # final layernorm, hiding the last AllReduce + LN tail of chunk 1.
        NPRE = 16

        def logit_block(vb, t2, ebt):
            pt = ps_mm.tile([128, TCH], F32, tag="mm",
                            name=f"plg{vb}_{t2}")
            for k in range(KD):
                nc.tensor.matmul(pt[:], ebt[:, k, :],
                                 nf[:, k, ts(t2, TCH)],
                                 start=(k == 0), stop=(k == KD - 1))
            lo = lout.tile([128, TCH], F32, tag="lo", name=f"lo{vb}_{t2}")
            nc.scalar.copy(lo[:], pt[:])
            nc.sync.dma_start(out=logits[ts(vb, 128), ts(t2, TCH)],
                              in_=lo[:])

        for vb in range(NPRE):
            ebt = embp.tile([128, KD, 128], F32R, tag="emb", name=f"ebA{vb}")
            nc.sync.dma_start(out=ebt[:], in_=inp["emb"][:, :, ts(vb, 128)])
            logit_block(vb, 0, ebt)
        for vb in range(VSP // 128):
            ebt = embp.tile([128, KD, 128], F32R, tag="emb", name=f"ebB{vb}")
            nc.sync.dma_start(out=ebt[:], in_=inp["emb"][:, :, ts(vb, 128)])
            for t2 in range(NTC):
                if t2 == 0 and vb < NPRE:
                    continue
                logit_block(vb, t2, ebt)


# ------------------------------------------------------------------
# Host side
# ------------------------------------------------------------------

def _kfold(w):
    """[in, out] -> [128, in//128, out] K-tiled layout."""
    i, o = w.shape
    return np.ascontiguousarray(
        w.reshape(i // 128, 128, o).transpose(1, 0, 2))


def _cols(v):
    """[n] -> [128, n//128] per-partition bias columns."""
    return np.ascontiguousarray(v.reshape(-1, 128).T)


def prep_inputs(inputs):
    """Full inputs -> list of 8 per-core input maps (numpy float32)."""
    f = lambda a: np.asarray(a, np.float32)
    tokens = np.asarray(inputs["tokens"])
    tok_emb, pos_emb = f(inputs["tok_emb"]), f(inputs["pos_emb"])
    ln1_g, ln1_b = f(inputs["ln1_g"]), f(inputs["ln1_b"])
    wq, wk = f(inputs["wq"]), f(inputs["wk"])
    wv, wo = f(inputs["wv"]), f(inputs["wo"])
    ln2_g, ln2_b = f(inputs["ln2_g"]), f(inputs["ln2_b"])
    w1, b1 = f(inputs["w1"]), f(inputs["b1"])
    w2, b2 = f(inputs["w2"]), f(inputs["b2"])
    lnf_g = f(inputs["lnf_g"])

    sc = 1.0 / np.sqrt(HD)
    x0 = tok_emb[tokens] + pos_emb[:S][None]          # [B, S, D]
    ones = np.ones((128, 1), np.float32)
    ident = np.eye(128, dtype=np.float32)
    tri = np.triu(np.full((128, 128), MASK_VAL, np.float32), k=1)

    in_maps = []
    for core in range(N_CORES):
        b = core // TP
        tpr = core % TP
        m = {
            "x0": _kfold(np.ascontiguousarray(x0[b].T)),
            "ones": ones, "ident": ident, "tri": tri,
            "ident2": np.vstack([np.eye(64), np.eye(64)]).astype(np.float32),
        }
        qs = slice(tpr * DQ, (tpr + 1) * DQ)
        fs = slice(tpr * FFC, (tpr + 1) * FFC)
        for l in range(L):
            wql = wq[l][:, qs] * sc
            wkl = wk[l][:, qs]
            wvl = wv[l][:, qs]
            m[_f("wq", l)] = _kfold(ln1_g[l][:, None] * wql)
            m[_f("wk", l)] = _kfold(ln1_g[l][:, None] * wkl)
            m[_f("wv", l)] = _kfold(ln1_g[l][:, None] * wvl)
            m[_f("wo", l)] = _kfold(wo[l][qs, :])
            m[_f("w1", l)] = _kfold(ln2_g[l][:, None] * w1[l][:, fs])
            m[_f("w2", l)] = _kfold(w2[l][fs, :])
            m[_f("bqkv", l)] = np.concatenate(
                [_cols(ln1_b[l] @ wql), _cols(ln1_b[l] @ wkl),
                 _cols(ln1_b[l] @ wvl)], axis=1)
            m[_f("b1", l)] = _cols(b1[l][fs] + ln2_b[l] @ w1[l][:, fs])
            m[_f("b2", l)] = _cols(b2[l])
        v0 = tpr * VS
        v1 = min(v0 + VS, V)
        epad = np.zeros((D, VSP), np.float32)
        epad[:, :v1 - v0] = (tok_emb[v0:v1] * lnf_g[None, :]).T
        m["emb"] = _kfold(epad)
        in_maps.append(m)
    return in_maps


_CACHED = {}


def _get_program(debug_taps=False):
    key = bool(debug_taps)
    if key not in _CACHED:
        _CACHED[key] = build_program(debug_taps)
    return _CACHED[key]


def run(inputs, debug_taps=False, trace=False, **kw):
    nc = _get_program(debug_taps)
    in_maps = prep_inputs(inputs)
    return run_bass_kernel_spmd(nc, in_maps, list(range(N_CORES)),
                                trace=trace, **kw)


def assemble(results, inputs):
    """Per-core logits -> full [B, S, V] float32."""
    lnf_b = np.asarray(inputs["lnf_b"], np.float32)
    tok_emb = np.asarray(inputs["tok_emb"], np.float32)
    out = np.empty((B, S, V), np.float32)
    for b in range(B):
        parts = []
        for tpr in range(TP):
            v0 = tpr * VS
            v1 = min(v0 + VS, V)
            parts.append(results[b * TP + tpr]["logits"][:v1 - v0, :].T)
        out[b] = np.concatenate(parts, axis=1)
    if np.any(lnf_b):
        out += (tok_emb @ lnf_b)[None, None, :]
    return out


def kernel(**inputs):
    res = run(inputs)
    return assemble(res.results, inputs)


if __name__ == "__main__":
    print("building program...")
    build_program()
    print("build + compile OK")



# revision 3
# speedup vs baseline: 1.5223x; 1.5223x over previous
"""GPT forward pass on 8 Trainium2 NeuronCores (Bass/Tile) — v2 bf16.

Model: B=2, S=1024, D=1024, H=16 heads (hd=64), L=6 layers, V=50257,
tied embedding head.

Sharding: DP2 x TP4. Cores 0-3 compute batch element 0, cores 4-7
batch element 1. Within each group of 4: attention is head-sharded
(4 heads/core), the FFN hidden dim is column/row sharded (1024/core),
and the tied logit matrix is vocab-sharded (12565 rows/core, padded
to 12800). Two AllReduces per layer (post-attention, post-FFN) over
each group of 4, token-chunked (2 x 512) and reordered so every AR
overlaps compute (FFN chunk 0 runs between the Wo-AR of chunk 1 and
the residual add that consumes it, etc).

v2 changes vs v1:
- All GEMM operands (weights + activations) in bf16: full PE rate at
  any moving width, half the SBUF/DMA/collective bytes. The residual
  stream and LN stats stay f32.
- AllReduce payloads bf16 (half the wire bytes; ~55 GB/s busbw).
- LN's rsqrt computed as exp(-0.5*ln(var+eps)): Ln and Exp live in the
  same Act table set as attention's Exp, so the only table switches
  are around the FFN Gelu (pre-warmed off the critical path).
- Logits emitted bf16 (host upcasts): halves the output writeback.
- PSUM->SBUF logit copies alternate Act/DVE to balance engine load.

On-device layout: activations are feature-major ("transposed"):
x[p, k, t] = X^T[128k + p, t]. All matmuls take weights [in, out] as
the stationary operand and activations [in, tokens] as the moving
operand. LayerNorm gains/biases and the attention 1/sqrt(hd) scale
are folded into the adjacent weights on the host.
"""

import sys

sys.path.insert(0, "/opt/trn_rl_repo")

import contextlib

import ml_dtypes
import numpy as np

import concourse.bacc as bacc
import concourse.mybir as mybir
import concourse.tile as tile
from concourse.bass import ts
from concourse.bass_utils import run_bass_kernel_spmd

F32 = mybir.dt.float32
F32R = mybir.dt.float32r
BF16 = mybir.dt.bfloat16
AF = mybir.ActivationFunctionType
ALU = mybir.AluOpType

# Model dims
B, S, D, H, L, V = 2, 1024, 1024, 16, 6, 50257
HD = D // H           # 64
DFF = 4 * D           # 4096
N_CORES = 8
TP = 4                # tensor-parallel group size
HPC = H // TP         # heads per core = 4
DQ = HPC * HD         # per-core qkv width = 256
FFC = DFF // TP       # per-core ffn hidden = 1024
KD = D // 128         # 8 feature tiles
T = S                 # tokens per core (one batch element)
TCH = 512             # token chunk for AR pipelining
NTC = T // TCH        # 2
VS = 12565            # vocab rows per core (last core: 12562)
VSP = 12800           # padded vocab rows per core
MASK_VAL = -60.0

REPLICA_GROUPS = [[0, 1, 2, 3], [4, 5, 6, 7]]


def _f(name, l=None):
    return name if l is None else f"{name}{l}"


def build_program():
    """Build the SPMD bass program (same instruction stream on all cores)."""
    nc = bacc.Bacc("TRN2", target_bir_lowering=False, debug=False,
                   enable_asserts=True, num_devices=N_CORES)

    inp = {}

    def dram_in(name, shape, dtype=BF16):
        inp[name] = nc.dram_tensor(name, shape, dtype, kind="ExternalInput").ap()
        return inp[name]

    dram_in("x0", [128, KD, T], F32R)
    dram_in("ones", [128, 1], F32R)
    dram_in("ones16", [128, 1])
    dram_in("ident", [128, 128])
    dram_in("ident2", [128, 64])
    dram_in("triT", [128, 128])
    for l in range(L):
        dram_in(_f("wq", l), [128, KD, DQ])
        dram_in(_f("wk", l), [128, KD, DQ])
        dram_in(_f("wv", l), [128, KD, DQ])
        dram_in(_f("wo", l), [128, DQ // 128, D])
        dram_in(_f("w1", l), [128, KD, FFC])
        dram_in(_f("w2", l), [128, FFC // 128, D])
        dram_in(_f("bqkv", l), [128, 6], F32)
        dram_in(_f("b1", l), [128, FFC // 128], F32)
        dram_in(_f("b2", l), [128, KD], F32)
    dram_in("emb", [128, KD, VSP])
    logits = nc.dram_tensor("logits", [VSP, T], BF16, kind="ExternalOutput").ap()

    with tile.TileContext(nc) as tc:
        _body(tc, inp, logits)
    nc.compile()
    return nc


def _body(tc, inp, logits):
    nc = tc.nc
    ctx = contextlib.ExitStack()
    with ctx:
        # --- SBUF pools ---
        singles = ctx.enter_context(tc.tile_pool(name="singles", bufs=1))
        xp = ctx.enter_context(tc.tile_pool(name="xp", bufs=1))        # x f32
        npool = ctx.enter_context(tc.tile_pool(name="npool", bufs=1))
        tmp = ctx.enter_context(tc.tile_pool(name="tmp", bufs=2))
        qkv = ctx.enter_context(tc.tile_pool(name="qkv", bufs=1))
        vh = ctx.enter_context(tc.tile_pool(name="vh", bufs=4))
        ep = ctx.enter_context(tc.tile_pool(name="ep", bufs=2))
        et = ctx.enter_context(tc.tile_pool(name="et", bufs=1))
        op = ctx.enter_context(tc.tile_pool(name="op", bufs=1))
        hp = ctx.enter_context(tc.tile_pool(name="hp", bufs=1))
        arr = ctx.enter_context(tc.tile_pool(name="arr", bufs=1))
        wts = ctx.enter_context(tc.tile_pool(name="wts", bufs=3))
        embp = ctx.enter_context(tc.tile_pool(name="embp", bufs=3))
        stat = ctx.enter_context(tc.tile_pool(name="stat", bufs=2))
        bcp = ctx.enter_context(tc.tile_pool(name="bcp", bufs=1))
        lout = ctx.enter_context(tc.tile_pool(name="lout", bufs=2))
        bias = ctx.enter_context(tc.tile_pool(name="bias", bufs=3))
        stg = ctx.enter_context(tc.tile_pool(name="stg", bufs=2))
        # --- PSUM pools (8 banks total) ---
        ps_mm = ctx.enter_context(tc.tile_pool(name="ps_mm", bufs=2, space="PSUM"))
        ps_st = ctx.enter_context(tc.tile_pool(name="ps_st", bufs=1, space="PSUM"))
        ps_sc = ctx.enter_context(tc.tile_pool(name="ps_sc", bufs=2, space="PSUM"))
        ps_tr = ctx.enter_context(tc.tile_pool(name="ps_tr", bufs=2, space="PSUM"))
        ps_o = ctx.enter_context(tc.tile_pool(name="ps_o", bufs=1, space="PSUM"))
        # --- DRAM (collective bounce) ---
        dram = ctx.enter_context(tc.tile_pool(name="dram", bufs=4, space="DRAM"))

        # --- constants / persistent ---
        ones_t = singles.tile([128, 1], F32R)
        nc.sync.dma_start(out=ones_t[:], in_=inp["ones"][:])
        ident_t = singles.tile([128, 128], BF16)
        nc.sync.dma_start(out=ident_t[:], in_=inp["ident"][:])
        ident2_t = singles.tile([128, 64], BF16)
        nc.sync.dma_start(out=ident2_t[:], in_=inp["ident2"][:])
        tri_t = singles.tile([128, 128], BF16)
        nc.sync.dma_start(out=tri_t[:], in_=inp["tri"][:])
        eps_t = singles.tile([1, 1], F32)
        nc.vector.memset(eps_t[:], 1e-5)
        warm_t = singles.tile([1, 1], F32)
        nc.vector.memset(warm_t[:], 1.0)
        warm_o = singles.tile([1, 1], F32)

        def act_warm(func):
            """Dummy activation to pre-load an Act table set while the
            Act engine is idle, off the critical path."""
            nc.scalar.activation(warm_o[:], warm_t[:], func)

        xt = xp.tile([128, KD, T], F32R, tag="x")
        nc.sync.dma_start(out=xt[:], in_=inp["x0"][:])

        def layer_norm_chunk(src, dst, c):
            """dst[:,:,c] = (src - mean) * rsqrt(var + eps); dst bf16."""
            cs = ts(c, TCH)
            s1 = ps_st.tile([1, TCH], F32, tag="st")
            s2 = ps_st.tile([1, TCH], F32, tag="st")
            for k in range(KD):
                nc.tensor.matmul(s1[:], ones_t[:], src[:, k, cs],
                                 start=(k == 0), stop=(k == KD - 1))
            for k in range(KD):
                sq = tmp.tile([128, TCH], F32R, tag="tmp")
                nc.vector.tensor_tensor(
                    out=sq[:], in0=src[:, k, cs].bitcast(F32),
                    in1=src[:, k, cs].bitcast(F32), op=ALU.mult)
                nc.tensor.matmul(s2[:], ones_t[:], sq[:],
                                 start=(k == 0), stop=(k == KD - 1))
            # finishing math on [1, TCH] rows
            m = stat.tile([1, TCH], F32, tag="sa")
            nc.vector.tensor_scalar_mul(m[:], s1[:], 1.0 / D)
            msq = stat.tile([1, TCH], F32, tag="sb")
            nc.vector.tensor_tensor(out=msq[:], in0=m[:], in1=m[:],
                                    op=ALU.mult)
            var = stat.tile([1, TCH], F32, tag="sb")
            nc.vector.scalar_tensor_tensor(
                out=var[:], in0=s2[:], scalar=1.0 / D, in1=msq[:],
                op0=ALU.mult, op1=ALU.subtract)
            # rsqrt(var+eps) = exp(-0.5 * ln(var+eps)) — Ln and Exp share
            # one Act table set with attention's Exp (no table switch).
            lnv = stat.tile([1, TCH], F32, tag="sb")
            nc.scalar.activation(lnv[:], var[:], AF.Ln, bias=eps_t[:])
            rs = stat.tile([1, TCH], F32, tag="sb")
            nc.scalar.activation(rs[:], lnv[:], AF.Exp, scale=-0.5)
            a = stat.tile([1, TCH], F32, tag="sa")
            nc.vector.scalar_tensor_tensor(
                out=a[:], in0=m[:], scalar=-1.0, in1=rs[:],
                op0=ALU.mult, op1=ALU.mult)
            rB = bcp.tile([128, TCH], F32, tag="rB")
            nc.gpsimd.partition_broadcast(rB[:], rs[:])
            aB = bcp.tile([128, TCH], F32, tag="aB")
            nc.gpsimd.partition_broadcast(aB[:], a[:])
            # dst = src * rB + aB   (two DVE passes per k)
            for k in range(KD):
                t2 = tmp.tile([128, TCH], F32, tag="tmp")
                nc.vector.tensor_tensor(
                    out=t2[:], in0=src[:, k, cs].bitcast(F32),
                    in1=rB[:], op=ALU.mult)
                nc.vector.tensor_tensor(
                    out=dst[:, k, cs], in0=t2[:], in1=aB[:], op=ALU.add)

        def proj_chunk(dst, wname, n_src, mchunks, bias_t, bcol0, c):
            """dst[:, m, c-chunk] (bf16) = W^T @ n_src[c-chunk] + bias."""
            cs = ts(c, TCH)
            for m in range(mchunks):
                wstrip = wts.tile([128, KD, 128], BF16, tag="w")
                nc.sync.dma_start(out=wstrip[:],
                                  in_=inp[wname][:, :, ts(m, 128)])
                pt = ps_mm.tile([128, TCH], F32, tag="mm")
                for k in range(KD):
                    nc.tensor.matmul(pt[:], wstrip[:, k, :],
                                     n_src[:, k, cs],
                                     start=(k == 0), stop=(k == KD - 1))
                nc.scalar.activation(
                    dst[:, m, cs], pt[:], AF.Identity,
                    bias=bias_t[:, bcol0 + m:bcol0 + m + 1])

        def qkv_head_chunk(l, c, n_src, qT, kT, vT, bqkv_t, vhs):
            """QKV projections for token chunk c, plus the v-transposes
            whose key blocks live in chunk c."""
            proj_chunk(qT, _f("wq", l), n_src, DQ // 128, bqkv_t, 0, c)
            proj_chunk(kT, _f("wk", l), n_src, DQ // 128, bqkv_t, 2, c)
            proj_chunk(vT, _f("wv", l), n_src, DQ // 128, bqkv_t, 4, c)
            for h in range(HPC):
                pp = 64 * (h % 2)
                mh = h // 2
                for j in range(4 * c, 4 * c + 4):
                    tp_ = ps_tr.tile([128, 128], BF16, tag="tr")
                    nc.tensor.matmul(tp_[:, 0:HD],
                                     vT[pp:pp + 64, mh, ts(j, 128)],
                                     ident2_t[pp:pp + 64, :],
                                     is_transpose=True,
                                     start=True, stop=True)
                    nc.scalar.copy(vhs[h][:, j, :], tp_[:, 0:HD])

        def attn_chunk(l, c, qT, kT, vhs, oT, inject=None, inject_at=1):
            for h in range(HPC):
                if h == inject_at and inject is not None:
                    inject()
                pp = 64 * (h % 2)
                mh = h // 2
                etile = et.tile([128, KD, TCH], BF16, tag="et")
                nkj = 4 * (c + 1)
                for qi in range(4 * c, 4 * c + 4):
                    qs = ts(qi, 128)
                    nkeys = 128 * (qi + 1)
                    erow = ep.tile([128, T], BF16, tag="e")
                    rsum = stat.tile([128, 1], F32, tag="rsum")
                    nchunks = (nkeys + 511) // 512
                    for sc in range(nchunks):
                        w = min(512, nkeys - 512 * sc)
                        last = sc == nchunks - 1
                        spt = ps_sc.tile([128, 512], F32, tag="sc")
                        nc.tensor.matmul(
                            spt[:, :w], qT[pp:pp + 64, mh, qs],
                            kT[pp:pp + 64, mh, 512 * sc:512 * sc + w],
                            start=True, stop=not last)
                        if last:
                            # add the causal mask for the diagonal block
                            nc.tensor.matmul(spt[:, w - 128:w], ident_t[:],
                                             tri_t[:], start=False, stop=True)
                        if sc == 0:
                            nc.scalar.activation(
                                erow[:, :w], spt[:, :w], AF.Exp,
                                accum_out=rsum[:])
                        else:
                            rpart = stat.tile([128, 1], F32, tag="rp")
                            nc.scalar.activation(
                                erow[:, 512 * sc:512 * sc + w],
                                spt[:, :w], AF.Exp, accum_out=rpart[:])
                            nc.vector.tensor_tensor(
                                out=rsum[:], in0=rsum[:], in1=rpart[:],
                                op=ALU.add)
                    nc.vector.reciprocal(rsum[:], rsum[:])
                    en = ep.tile([128, T], BF16, tag="e")
                    nc.scalar.activation(en[:, :nkeys], erow[:, :nkeys],
                                         AF.Copy, scale=rsum[:])
                    for kj in range(qi + 1):
                        tp_ = ps_tr.tile([128, 128], BF16, tag="tr")
                        nc.tensor.matmul(tp_[:], en[:, ts(kj, 128)],
                                         ident_t[:], is_transpose=True,
                                         start=True, stop=True)
                        qo = 128 * (qi - 4 * c)
                        nc.scalar.copy(etile[:, kj, qo:qo + 128], tp_[:])
                po = ps_o.tile([64, TCH], F32, tag="o")
                for kj in range(nkj):
                    lo = max(0, 128 * kj - TCH * c)
                    nc.tensor.matmul(po[:, lo:TCH], vhs[h][:, kj, :],
                                     etile[:, kj, lo:TCH],
                                     start=(kj == 0), stop=(kj == nkj - 1))
                nc.scalar.copy(oT[pp:pp + 64, mh, ts(c, TCH)], po[:])

        def mm_ar_chunk(wname, kchunks, src_tile, c, src_is_chunk):
            """out-partial = W^T @ src for chunk c -> bf16 AllReduce."""
            ar_in = dram.tile([128, KD, TCH], BF16, tag="dr")
            ar_out = dram.tile([128, KD, TCH], BF16, tag="dr")
            for m in range(KD):
                wstrip = wts.tile([128, KD, 128], BF16, tag="w")
                nc.sync.dma_start(out=wstrip[:, 0:kchunks, :],
                                  in_=inp[wname][:, :, ts(m, 128)])
                pt = ps_mm.tile([128, TCH], F32, tag="mm")
                for k in range(kchunks):
                    s = (src_tile[:, k, :] if src_is_chunk
                         else src_tile[:, k, ts(c, TCH)])
                    nc.tensor.matmul(pt[:], wstrip[:, k, :], s,
                                     start=(k == 0), stop=(k == kchunks - 1))
                st_ = stg.tile([128, TCH], BF16, tag="stg")
                nc.scalar.copy(st_[:], pt[:])
                nc.sync.dma_start(out=ar_in[:, m, :], in_=st_[:])
            nc.gpsimd.collective_compute(
                "AllReduce", ALU.add, replica_groups=REPLICA_GROUPS,
                ins=[ar_in.opt()], outs=[ar_out.opt()])
            return ar_out

        # ---------------- prologue: LN1 + QKV of layer 0 ----------------
        n_cur = npool.tile([128, KD, T], BF16, tag="n")
        bqkv_t = bias.tile([128, 6], F32, tag="bias")
        nc.sync.dma_start(out=bqkv_t[:], in_=inp[_f("bqkv", 0)][:])
        qT = qkv.tile([128, DQ // 128, T], BF16, tag="qT")
        kT = qkv.tile([128, DQ // 128, T], BF16, tag="kT")
        vT = qkv.tile([128, DQ // 128, T], BF16, tag="vT")
        vhs = [vh.tile([128, KD, HD], BF16, tag="vh", name=f"vh{i}")
               for i in range(HPC)]
        for c in range(NTC):
            layer_norm_chunk(xt, n_cur, c)
            qkv_head_chunk(0, c, n_cur, qT, kT, vT, bqkv_t, vhs)

        def ffn_w1(l, c, n2, b1_t):
            cs = ts(c, TCH)
            hT = hp.tile([128, FFC // 128, TCH], BF16, tag="h",
                         name=f"hT{l}_{c}")
            for m in range(FFC // 128):
                wstrip = wts.tile([128, KD, 128], BF16, tag="w",
                                  name=f"w1s{l}_{c}_{m}")
                nc.sync.dma_start(out=wstrip[:],
                                  in_=inp[_f("w1", l)][:, :, ts(m, 128)])
                pt = ps_mm.tile([128, TCH], F32, tag="mm", name=f"p1_{l}_{c}_{m}")
                for k in range(KD):
                    nc.tensor.matmul(pt[:], wstrip[:, k, :], n2[:, k, cs],
                                     start=(k == 0), stop=(k == KD - 1))
                nc.scalar.activation(hT[:, m, :], pt[:], AF.Gelu,
                                     bias=b1_t[:, m:m + 1])
            # pre-warm the Ln/Exp table set while Act is idle so the next
            # layer-norm doesn't pay the switch on the critical path
            act_warm(AF.Ln)
            return hT

        def add_f_lnnext(c, ar_f, b2_t, n_next):
            cs = ts(c, TCH)
            art = arr.tile([128, KD, TCH], BF16, tag="arr", name=f"artf{c}")
            nc.sync.dma_start(out=art[:], in_=ar_f[c][:])
            for m in range(KD):
                nc.vector.scalar_tensor_tensor(
                    out=xt[:, m, cs], in0=art[:, m, :],
                    scalar=b2_t[:, m:m + 1],
                    in1=xt[:, m, cs].bitcast(F32),
                    op0=ALU.add, op1=ALU.add)
            layer_norm_chunk(xt, n_next, c)

        for l in range(L):
            # ------- attention + Wo + AR, chunk-pipelined -------
            oT = op.tile([128, DQ // 128, T], BF16, tag="oT")
            n2 = npool.tile([128, KD, T], BF16, tag="n")
            ar_a = []

            def add_a(c):
                art = arr.tile([128, KD, TCH], BF16, tag="arr",
                               name=f"arta{c}_{l}")
                nc.sync.dma_start(out=art[:], in_=ar_a[c][:])
                nc.vector.tensor_tensor(
                    out=xt[:, :, ts(c, TCH)],
                    in0=xt[:, :, ts(c, TCH)].bitcast(F32),
                    in1=art[:], op=ALU.add)

            def add_a_ln2_c0():
                add_a(0)
                layer_norm_chunk(xt, n2, 0)

            attn_chunk(l, 0, qT, kT, vhs, oT)
            ar_a.append(mm_ar_chunk(_f("wo", l), DQ // 128, oT, 0, False))
            attn_chunk(l, 1, qT, kT, vhs, oT, inject=add_a_ln2_c0)
            ar_a.append(mm_ar_chunk(_f("wo", l), DQ // 128, oT, 1, False))

            # ------- FFN chunk 0 first: covers the in-flight AR_a1 -------
            b1_t = bias.tile([128, FFC // 128], F32, tag="bias")
            nc.sync.dma_start(out=b1_t[:], in_=inp[_f("b1", l)][:])
            b2_t = bias.tile([128, KD], F32, tag="bias")
            nc.sync.dma_start(out=b2_t[:], in_=inp[_f("b2", l)][:])

            ar_f = []
            hT0 = ffn_w1(l, 0, n2, b1_t)
            ar_f.append(mm_ar_chunk(_f("w2", l), FFC // 128, hT0, 0, True))
            # residual + LN2 for chunk 1 (needs AR_a1, now covered)
            add_a(1)
            layer_norm_chunk(xt, n2, 1)
            hT1 = ffn_w1(l, 1, n2, b1_t)

            n_next = npool.tile([128, KD, T], BF16, tag="n")
            ar_f.append(mm_ar_chunk(_f("w2", l), FFC // 128, hT1, 1, True))
            add_f_lnnext(0, ar_f, b2_t, n_next)

            if l < L - 1:
                bqkv_t = bias.tile([128, 6], F32, tag="bias")
                nc.sync.dma_start(out=bqkv_t[:], in_=inp[_f("bqkv", l + 1)][:])
                qT = qkv.tile([128, DQ // 128, T], BF16, tag="qT")
                kT = qkv.tile([128, DQ // 128, T], BF16, tag="kT")
                vT = qkv.tile([128, DQ // 128, T], BF16, tag="vT")
                vhs = [vh.tile([128, KD, HD], BF16, tag="vh",
                               name=f"vh{l + 1}_{i}") for i in range(HPC)]
                qkv_head_chunk(l + 1, 0, n_next, qT, kT, vT, bqkv_t, vhs)
            add_f_lnnext(1, ar_f, b2_t, n_next)
            if l < L - 1:
                qkv_head_chunk(l + 1, 1, n_next, qT, kT, vT, bqkv_t, vhs)
            n_cur = n_next

        # after the loop, n_cur holds the final layer norm
        nf = n_cur

        # logits^T [VSP, T]: emb strip is the stationary operand, nf moves.
        # A t2=0-only prefix runs first: it depends only on chunk 0 of the
        # final layernorm, hiding the last AllReduce + LN tail of chunk 1.
        NPRE = 16

        def logit_block(vb, t2, ebt, use_dve):
            pt = ps_mm.tile([128, TCH], F32, tag="mm",
                            name=f"plg{vb}_{t2}")
            for k in range(KD):
                nc.tensor.matmul(pt[:], ebt[:, k, :],
                                 nf[:, k, ts(t2, TCH)],
                                 start=(k == 0), stop=(k == KD - 1))
            lo = lout.tile([128, TCH], BF16, tag="lo", name=f"lo{vb}_{t2}")
            if use_dve:
                nc.vector.tensor_scalar_mul(lo[:], pt[:], 1.0)
            else:
                nc.scalar.copy(lo[:], pt[:])
            nc.sync.dma_start(out=logits[ts(vb, 128), ts(t2, TCH)],
                              in_=lo[:])

        for vb in range(NPRE):
            ebt = embp.tile([128, KD, 128], BF16, tag="emb", name=f"ebA{vb}")
            nc.sync.dma_start(out=ebt[:], in_=inp["emb"][:, :, ts(vb, 128)])
            logit_block(vb, 0, ebt, vb % 2 == 0)
        for vb in range(VSP // 128):
            ebt = embp.tile([128, KD, 128], BF16, tag="emb", name=f"ebB{vb}")
            nc.sync.dma_start(out=ebt[:], in_=inp["emb"][:, :, ts(vb, 128)])
            for t2 in range(NTC):
                if t2 == 0 and vb < NPRE:
                    continue
                logit_block(vb, t2, ebt, vb % 2 == 0)


# ------------------------------------------------------------------
# Host side
# ------------------------------------------------------------------

def _kfold(w):
    """[in, out] -> [128, in//128, out] K-tiled layout."""
    i, o = w.shape
    return np.ascontiguousarray(
        w.reshape(i // 128, 128, o).transpose(1, 0, 2))


def _bf(w):
    return np.ascontiguousarray(w).astype(ml_dtypes.bfloat16)


def _cols(v):
    """[n] -> [128, n//128] per-partition bias columns."""
    return np.ascontiguousarray(v.reshape(-1, 128).T)


def prep_inputs(inputs):
    """Full inputs -> list of 8 per-core input maps."""
    f = lambda a: np.asarray(a, np.float32)
    tokens = np.asarray(inputs["tokens"])
    tok_emb, pos_emb = f(inputs["tok_emb"]), f(inputs["pos_emb"])
    ln1_g, ln1_b = f(inputs["ln1_g"]), f(inputs["ln1_b"])
    wq, wk = f(inputs["wq"]), f(inputs["wk"])
    wv, wo = f(inputs["wv"]), f(inputs["wo"])
    ln2_g, ln2_b = f(inputs["ln2_g"]), f(inputs["ln2_b"])
    w1, b1 = f(inputs["w1"]), f(inputs["b1"])
    w2, b2 = f(inputs["w2"]), f(inputs["b2"])
    lnf_g = f(inputs["lnf_g"])

    sc = 1.0 / np.sqrt(HD)
    x0 = tok_emb[tokens] + pos_emb[:S][None]          # [B, S, D]
    ones = np.ones((128, 1), np.float32)
    ident = np.eye(128, dtype=np.float32)
    tri = np.triu(np.full((128, 128), MASK_VAL, np.float32), k=1)

    in_maps = []
    for core in range(N_CORES):
        b = core // TP
        tpr = core % TP
        m = {
            "x0": _kfold(np.ascontiguousarray(x0[b].T)),
            "ones": ones, "ident": _bf(ident), "tri": _bf(tri),
            "ident2": _bf(np.vstack([np.eye(64), np.eye(64)]).astype(np.float32)),
        }
        qs = slice(tpr * DQ, (tpr + 1) * DQ)
        fs = slice(tpr * FFC, (tpr + 1) * FFC)
        for l in range(L):
            wql = wq[l][:, qs] * sc
            wkl = wk[l][:, qs]
            wvl = wv[l][:, qs]
            m[_f("wq", l)] = _bf(_kfold(ln1_g[l][:, None] * wql))
            m[_f("wk", l)] = _bf(_kfold(ln1_g[l][:, None] * wkl))
            m[_f("wv", l)] = _bf(_kfold(ln1_g[l][:, None] * wvl))
            m[_f("wo", l)] = _bf(_kfold(wo[l][qs, :]))
            m[_f("w1", l)] = _bf(_kfold(ln2_g[l][:, None] * w1[l][:, fs]))
            m[_f("w2", l)] = _bf(_kfold(w2[l][fs, :]))
            m[_f("bqkv", l)] = np.concatenate(
                [_cols(ln1_b[l] @ wql), _cols(ln1_b[l] @ wkl),
                 _cols(ln1_b[l] @ wvl)], axis=1)
            m[_f("b1", l)] = _cols(b1[l][fs] + ln2_b[l] @ w1[l][:, fs])
            m[_f("b2", l)] = _cols(b2[l])
        v0 = tpr * VS
        v1 = min(v0 + VS, V)
        epad = np.zeros((D, VSP), np.float32)
        epad[:, :v1 - v0] = (tok_emb[v0:v1] * lnf_g[None, :]).T
        m["emb"] = _bf(_kfold(epad))
        in_maps.append(m)
    return in_maps


_CACHED = {}


def _get_program():
    if "nc" not in _CACHED:
        _CACHED["nc"] = build_program()
    return _CACHED["nc"]


def run(inputs, trace=False, **kw):
    nc = _get_program()
    in_maps = prep_inputs(inputs)
    return run_bass_kernel_spmd(nc, in_maps, list(range(N_CORES)),
                                trace=trace, **kw)


def assemble(results, inputs):
    """Per-core logits -> full [B, S, V] float32."""
    lnf_b = np.asarray(inputs["lnf_b"], np.float32)
    tok_emb = np.asarray(inputs["tok_emb"], np.float32)
    out = np.empty((B, S, V), np.float32)
    for b in range(B):
        parts = []
        for tpr in range(TP):
            v0 = tpr * VS
            v1 = min(v0 + VS, V)
            parts.append(
                results[b * TP + tpr]["logits"][:v1 - v0, :]
                .astype(np.float32).T)
        out[b] = np.concatenate(parts, axis=1)
    if np.any(lnf_b):
        out += (tok_emb @ lnf_b)[None, None, :]
    return out


def kernel(**inputs):
    res = run(inputs)
    return assemble(res.results, inputs)


if __name__ == "__main__":
    print("building program...")
    build_program()
    print("build + compile OK")


# revision 8
# speedup vs baseline: 1.7022x; 1.1182x over previous
"""GPT forward pass on 8 Trainium2 NeuronCores (Bass/Tile) — v2 bf16.

Model: B=2, S=1024, D=1024, H=16 heads (hd=64), L=6 layers, V=50257,
tied embedding head.

Sharding: DP2 x TP4. Cores 0-3 compute batch element 0, cores 4-7
batch element 1. Within each group of 4: attention is head-sharded
(4 heads/core), the FFN hidden dim is column/row sharded (1024/core),
and the tied logit matrix is vocab-sharded (12565 rows/core, padded
to 12800). Two AllReduces per layer (post-attention, post-FFN) over
each group of 4, token-chunked (2 x 512) and reordered so every AR
overlaps compute (FFN chunk 0 runs between the Wo-AR of chunk 1 and
the residual add that consumes it, etc).

v2 changes vs v1:
- All GEMM operands (weights + activations) in bf16: full PE rate at
  any moving width, half the SBUF/DMA/collective bytes. The residual
  stream and LN stats stay f32.
- AllReduce payloads bf16 (half the wire bytes; ~55 GB/s busbw).
- LN's rsqrt computed as exp(-0.5*ln(var+eps)): Ln and Exp live in the
  same Act table set as attention's Exp, so the only table switches
  are around the FFN Gelu (pre-warmed off the critical path).
- Logits emitted bf16 (host upcasts): halves the output writeback.
- PSUM->SBUF logit copies alternate Act/DVE to balance engine load.

On-device layout: activations are feature-major ("transposed"):
x[p, k, t] = X^T[128k + p, t]. All matmuls take weights [in, out] as
the stationary operand and activations [in, tokens] as the moving
operand. LayerNorm gains/biases and the attention 1/sqrt(hd) scale
are folded into the adjacent weights on the host.
"""

import sys

sys.path.insert(0, "/opt/trn_rl_repo")

import contextlib

import ml_dtypes
import numpy as np

import concourse.bacc as bacc
import concourse.mybir as mybir
import concourse.tile as tile
from concourse.bass import ts
from concourse.bass_utils import run_bass_kernel_spmd

F32 = mybir.dt.float32
F32R = mybir.dt.float32r
BF16 = mybir.dt.bfloat16
AF = mybir.ActivationFunctionType
ALU = mybir.AluOpType

# Model dims
B, S, D, H, L, V = 2, 1024, 1024, 16, 6, 50257
HD = D // H           # 64
DFF = 4 * D           # 4096
N_CORES = 8
TP = 4                # tensor-parallel group size
HPC = H // TP         # heads per core = 4
DQ = HPC * HD         # per-core qkv width = 256
FFC = DFF // TP       # per-core ffn hidden = 1024
KD = D // 128         # 8 feature tiles
T = S                 # tokens per core (one batch element)
TCH = 512             # token chunk for AR pipelining
NTC = T // TCH        # 2
VS = 12565            # vocab rows per core (last core: 12562)
VSP = 12800           # padded vocab rows per core
MASK_VAL = -60.0

REPLICA_GROUPS = [[0, 1, 2, 3], [4, 5, 6, 7]]


def _f(name, l=None):
    return name if l is None else f"{name}{l}"


def build_program():
    """Build the SPMD bass program (same instruction stream on all cores)."""
    nc = bacc.Bacc("TRN2", target_bir_lowering=False, debug=False,
                   enable_asserts=True, num_devices=N_CORES)

    inp = {}

    def dram_in(name, shape, dtype=BF16):
        inp[name] = nc.dram_tensor(name, shape, dtype, kind="ExternalInput").ap()
        return inp[name]

    dram_in("x0", [128, KD, T], F32R)
    dram_in("ones", [128, 1], F32R)
    dram_in("ones16", [128, 1])
    dram_in("ident", [128, 128])
    dram_in("ident2", [128, 64])
    dram_in("triT", [128, 128])
    for l in range(L):
        dram_in(_f("wq", l), [128, KD, DQ])
        dram_in(_f("wk", l), [128, KD, DQ])
        dram_in(_f("wv", l), [128, KD, DQ])
        dram_in(_f("wo", l), [128, DQ // 128, D])
        dram_in(_f("w1", l), [128, KD, FFC])
        dram_in(_f("w2", l), [128, FFC // 128, D])
        dram_in(_f("bqkv", l), [128, 6], F32)
        dram_in(_f("b1", l), [128, FFC // 128], F32)
        dram_in(_f("b2", l), [128, KD], F32)
    dram_in("emb", [128, KD, VSP])
    logits = nc.dram_tensor("logits", [VSP, T], BF16, kind="ExternalOutput").ap()

    with tile.TileContext(nc) as tc:
        _body(tc, inp, logits)
    nc.compile()
    return nc


def _body(tc, inp, logits):
    nc = tc.nc
    ctx = contextlib.ExitStack()
    with ctx:
        # --- SBUF pools ---
        singles = ctx.enter_context(tc.tile_pool(name="singles", bufs=1))
        xp = ctx.enter_context(tc.tile_pool(name="xp", bufs=1))        # x f32
        npool = ctx.enter_context(tc.tile_pool(name="npool", bufs=1))
        tmp = ctx.enter_context(tc.tile_pool(name="tmp", bufs=2))
        qkv = ctx.enter_context(tc.tile_pool(name="qkv", bufs=1))
        vh = ctx.enter_context(tc.tile_pool(name="vh", bufs=4))
        ep = ctx.enter_context(tc.tile_pool(name="ep", bufs=3))
        op = ctx.enter_context(tc.tile_pool(name="op", bufs=1))
        hp = ctx.enter_context(tc.tile_pool(name="hp", bufs=1))
        arr = ctx.enter_context(tc.tile_pool(name="arr", bufs=1))
        wts = ctx.enter_context(tc.tile_pool(name="wts", bufs=3))
        embp = ctx.enter_context(tc.tile_pool(name="embp", bufs=3))
        stat = ctx.enter_context(tc.tile_pool(name="stat", bufs=2))
        bcp = ctx.enter_context(tc.tile_pool(name="bcp", bufs=1))
        lout = ctx.enter_context(tc.tile_pool(name="lout", bufs=2))
        bias = ctx.enter_context(tc.tile_pool(name="bias", bufs=3))
        stg = ctx.enter_context(tc.tile_pool(name="stg", bufs=2))
        # --- PSUM pools (8 banks total) ---
        ps_mm = ctx.enter_context(tc.tile_pool(name="ps_mm", bufs=2, space="PSUM"))
        ps_st = ctx.enter_context(tc.tile_pool(name="ps_st", bufs=1, space="PSUM"))
        ps_sc = ctx.enter_context(tc.tile_pool(name="ps_sc", bufs=2, space="PSUM"))
        ps_tr = ctx.enter_context(tc.tile_pool(name="ps_tr", bufs=1, space="PSUM"))
        ps_o = ctx.enter_context(tc.tile_pool(name="ps_o", bufs=1, space="PSUM"))
        ps_rs = ctx.enter_context(tc.tile_pool(name="ps_rs", bufs=1, space="PSUM"))
        # --- DRAM (collective bounce) ---
        dram = ctx.enter_context(tc.tile_pool(name="dram", bufs=4, space="DRAM"))

        # --- constants / persistent ---
        ones_t = singles.tile([128, 1], F32R)
        nc.sync.dma_start(out=ones_t[:], in_=inp["ones"][:])
        ones16_t = singles.tile([128, 1], BF16)
        nc.sync.dma_start(out=ones16_t[:], in_=inp["ones16"][:])
        ident_t = singles.tile([128, 128], BF16)
        nc.sync.dma_start(out=ident_t[:], in_=inp["ident"][:])
        ident2_t = singles.tile([128, 64], BF16)
        nc.sync.dma_start(out=ident2_t[:], in_=inp["ident2"][:])
        triT_t = singles.tile([128, 128], BF16)
        nc.sync.dma_start(out=triT_t[:], in_=inp["triT"][:])
        eps_t = singles.tile([1, 1], F32)
        nc.vector.memset(eps_t[:], 1e-5)
        warm_t = singles.tile([1, 1], F32)
        nc.vector.memset(warm_t[:], 1.0)
        warm_o = singles.tile([1, 1], F32)

        def act_warm(func):
            """Dummy activation to pre-load an Act table set while the
            Act engine is idle, off the critical path."""
            nc.scalar.activation(warm_o[:], warm_t[:], func)

        xt = xp.tile([128, KD, T], F32R, tag="x")
        nc.sync.dma_start(out=xt[:], in_=inp["x0"][:])

        def layer_norm_chunk(src, dst, c):
            """dst[:,:,c] = (src - mean) * rsqrt(var + eps); dst bf16."""
            cs = ts(c, TCH)
            s1 = ps_st.tile([1, TCH], F32, tag="st")
            s2 = ps_st.tile([1, TCH], F32, tag="st")
            for k in range(KD):
                nc.tensor.matmul(s1[:], ones_t[:], src[:, k, cs],
                                 start=(k == 0), stop=(k == KD - 1))
            for k in range(KD):
                sq = tmp.tile([128, TCH], F32R, tag="tmp")
                nc.vector.tensor_tensor(
                    out=sq[:], in0=src[:, k, cs].bitcast(F32),
                    in1=src[:, k, cs].bitcast(F32), op=ALU.mult)
                nc.tensor.matmul(s2[:], ones_t[:], sq[:],
                                 start=(k == 0), stop=(k == KD - 1))
            # finishing math on [1, TCH] rows
            m = stat.tile([1, TCH], F32, tag="sa")
            nc.vector.tensor_scalar_mul(m[:], s1[:], 1.0 / D)
            msq = stat.tile([1, TCH], F32, tag="sb")
            nc.vector.tensor_tensor(out=msq[:], in0=m[:], in1=m[:],
                                    op=ALU.mult)
            var = stat.tile([1, TCH], F32, tag="sb")
            nc.vector.scalar_tensor_tensor(
                out=var[:], in0=s2[:], scalar=1.0 / D, in1=msq[:],
                op0=ALU.mult, op1=ALU.subtract)
            # rsqrt(var+eps) = exp(-0.5 * ln(var+eps)) — Ln and Exp share
            # one Act table set with attention's Exp (no table switch).
            lnv = stat.tile([1, TCH], F32, tag="sb")
            nc.scalar.activation(lnv[:], var[:], AF.Ln, bias=eps_t[:])
            rs = stat.tile([1, TCH], F32, tag="sb")
            nc.scalar.activation(rs[:], lnv[:], AF.Exp, scale=-0.5)
            a = stat.tile([1, TCH], F32, tag="sa")
            nc.vector.scalar_tensor_tensor(
                out=a[:], in0=m[:], scalar=-1.0, in1=rs[:],
                op0=ALU.mult, op1=ALU.mult)
            rB = bcp.tile([128, TCH], F32, tag="rB")
            nc.gpsimd.partition_broadcast(rB[:], rs[:])
            aB = bcp.tile([128, TCH], F32, tag="aB")
            nc.gpsimd.partition_broadcast(aB[:], a[:])
            # dst = src * rB + aB   (two DVE passes per k)
            for k in range(KD):
                t2 = tmp.tile([128, TCH], F32, tag="tmp")
                nc.vector.tensor_tensor(
                    out=t2[:], in0=src[:, k, cs].bitcast(F32),
                    in1=rB[:], op=ALU.mult)
                nc.vector.tensor_tensor(
                    out=dst[:, k, cs], in0=t2[:], in1=aB[:], op=ALU.add)

        def proj_chunk(dst, wname, n_src, mchunks, bias_t, bcol0, c):
            """dst[:, m, c-chunk] (bf16) = W^T @ n_src[c-chunk] + bias."""
            cs = ts(c, TCH)
            for m in range(mchunks):
                wstrip = wts.tile([128, KD, 128], BF16, tag="w")
                nc.sync.dma_start(out=wstrip[:],
                                  in_=inp[wname][:, :, ts(m, 128)])
                pt = ps_mm.tile([128, TCH], F32, tag="mm")
                for k in range(KD):
                    nc.tensor.matmul(pt[:], wstrip[:, k, :],
                                     n_src[:, k, cs],
                                     start=(k == 0), stop=(k == KD - 1))
                nc.scalar.activation(
                    dst[:, m, cs], pt[:], AF.Identity,
                    bias=bias_t[:, bcol0 + m:bcol0 + m + 1])

        def qkv_head_chunk(l, c, n_src, qT, kT, vT, bqkv_t, vhs):
            """QKV projections for token chunk c, plus the v-transposes
            whose key blocks live in chunk c."""
            proj_chunk(qT, _f("wq", l), n_src, DQ // 128, bqkv_t, 0, c)
            proj_chunk(kT, _f("wk", l), n_src, DQ // 128, bqkv_t, 2, c)
            proj_chunk(vT, _f("wv", l), n_src, DQ // 128, bqkv_t, 4, c)
            for h in range(HPC):
                pp = 64 * (h % 2)
                mh = h // 2
                for j in range(4 * c, 4 * c + 4):
                    tp_ = ps_tr.tile([128, 128], BF16, tag="tr")
                    nc.tensor.matmul(tp_[:, 0:HD],
                                     vT[pp:pp + 64, mh, ts(j, 128)],
                                     ident2_t[pp:pp + 64, :],
                                     is_transpose=True,
                                     start=True, stop=True)
                    nc.scalar.copy(vhs[h][:, j, :], tp_[:, 0:HD])

        def attn_chunk(l, c, qT, kT, vhs, oT, inject=None, inject_at=1):
            """Attention for token chunk c, scores computed transposed.

            sT[k, q] comes straight from a stationary-k / moving-q matmul,
            so exp(sT) lands in SBUF already in the layout the AV matmul
            consumes — no per-block PE transposes or PSUM->SBUF copies.
            Softmax sums come from a ones-matmul accumulated across key
            blocks; the normalization is folded into the single
            PSUM->SBUF pass of the attention output (per-token column
            scale via a GpSimd row broadcast + DVE multiply).
            """
            nkj = 4 * (c + 1)
            for h in range(HPC):
                if h == inject_at and inject is not None:
                    inject()
                pp = 64 * (h % 2)
                mh = h // 2
                po = ps_o.tile([64, TCH], F32, tag="o")
                rsp = ps_rs.tile([1, TCH], F32, tag="rs")

                def score_exp(kj):
                    lo = max(0, 128 * kj - TCH * c)
                    diag = 128 * kj >= TCH * c
                    spt = ps_sc.tile([128, TCH], F32, tag="sc")
                    nc.tensor.matmul(
                        spt[:, lo:TCH],
                        kT[pp:pp + 64, mh, ts(kj, 128)],
                        qT[pp:pp + 64, mh, TCH * c + lo:TCH * (c + 1)],
                        start=True, stop=not diag)
                    if diag:
                        # additive causal mask on the diagonal block
                        nc.tensor.matmul(spt[:, lo:lo + 128], ident_t[:],
                                         triT_t[:], start=False, stop=True)
                    eT = ep.tile([128, TCH], BF16, tag="e")
                    nc.scalar.activation(eT[:, lo:TCH], spt[:, lo:TCH],
                                         AF.Exp)
                    return eT

                def sum_av(kj, eT):
                    lo = max(0, 128 * kj - TCH * c)
                    nc.tensor.matmul(rsp[:, lo:TCH], ones16_t[:],
                                     eT[:, lo:TCH],
                                     start=(kj == 0), stop=(kj == nkj - 1))
                    nc.tensor.matmul(po[:, lo:TCH], vhs[h][:, kj, :],
                                     eT[:, lo:TCH],
                                     start=(kj == 0), stop=(kj == nkj - 1))

                # software-pipelined: score/exp of block kj+1 overlaps the
                # rsum/AV matmuls consuming block kj
                prev = score_exp(0)
                for kj in range(1, nkj):
                    cur = score_exp(kj)
                    sum_av(kj - 1, prev)
                    prev = cur
                sum_av(nkj - 1, prev)

                rinv = stat.tile([1, TCH], F32, tag="ri")
                nc.vector.reciprocal(rinv[:], rsp[:])
                rB = bcp.tile([128, TCH], F32, tag="rA")
                nc.gpsimd.partition_broadcast(rB[:], rinv[:])
                nc.vector.tensor_tensor(
                    out=oT[pp:pp + 64, mh, ts(c, TCH)], in0=po[:],
                    in1=rB[0:64, :], op=ALU.mult)

        def mm_ar_chunk(wname, kchunks, src_tile, c, src_is_chunk):
            """out-partial = W^T @ src for chunk c -> bf16 AllReduce."""
            ar_in = dram.tile([128, KD, TCH], BF16, tag="dr")
            ar_out = dram.tile([128, KD, TCH], BF16, tag="dr")
            for m in range(KD):
                wstrip = wts.tile([128, KD, 128], BF16, tag="w")
                nc.sync.dma_start(out=wstrip[:, 0:kchunks, :],
                                  in_=inp[wname][:, :, ts(m, 128)])
                pt = ps_mm.tile([128, TCH], F32, tag="mm")
                for k in range(kchunks):
                    s = (src_tile[:, k, :] if src_is_chunk
                         else src_tile[:, k, ts(c, TCH)])
                    nc.tensor.matmul(pt[:], wstrip[:, k, :], s,
                                     start=(k == 0), stop=(k == kchunks - 1))
                st_ = stg.tile([128, TCH], BF16, tag="stg")
                nc.scalar.copy(st_[:], pt[:])
                nc.sync.dma_start(out=ar_in[:, m, :], in_=st_[:])
            nc.gpsimd.collective_compute(
                "AllReduce", ALU.add, replica_groups=REPLICA_GROUPS,
                ins=[ar_in.opt()], outs=[ar_out.opt()])
            return ar_out

        # ---------------- prologue: LN1 + QKV of layer 0 ----------------
        n_cur = npool.tile([128, KD, T], BF16, tag="n")
        bqkv_t = bias.tile([128, 6], F32, tag="bias")
        nc.sync.dma_start(out=bqkv_t[:], in_=inp[_f("bqkv", 0)][:])
        qT = qkv.tile([128, DQ // 128, T], BF16, tag="qT")
        kT = qkv.tile([128, DQ // 128, T], BF16, tag="kT")
        vT = qkv.tile([128, DQ // 128, T], BF16, tag="vT")
        vhs = [vh.tile([128, KD, HD], BF16, tag="vh", name=f"vh{i}")
               for i in range(HPC)]
        for c in range(NTC):
            layer_norm_chunk(xt, n_cur, c)
            qkv_head_chunk(0, c, n_cur, qT, kT, vT, bqkv_t, vhs)

        def ffn_w1(l, c, n2, b1_t):
            cs = ts(c, TCH)
            hT = hp.tile([128, FFC // 128, TCH], BF16, tag="h",
                         name=f"hT{l}_{c}")
            for m in range(FFC // 128):
                wstrip = wts.tile([128, KD, 128], BF16, tag="w",
                                  name=f"w1s{l}_{c}_{m}")
                nc.sync.dma_start(out=wstrip[:],
                                  in_=inp[_f("w1", l)][:, :, ts(m, 128)])
                pt = ps_mm.tile([128, TCH], F32, tag="mm", name=f"p1_{l}_{c}_{m}")
                for k in range(KD):
                    nc.tensor.matmul(pt[:], wstrip[:, k, :], n2[:, k, cs],
                                     start=(k == 0), stop=(k == KD - 1))
                nc.scalar.activation(hT[:, m, :], pt[:], AF.Gelu,
                                     bias=b1_t[:, m:m + 1])
            # pre-warm the Ln/Exp table set while Act is idle so the next
            # layer-norm doesn't pay the switch on the critical path
            act_warm(AF.Ln)
            return hT

        def add_f_lnnext(c, ar_f, b2_t, n_next):
            cs = ts(c, TCH)
            art = arr.tile([128, KD, TCH], BF16, tag="arr", name=f"artf{c}")
            nc.sync.dma_start(out=art[:], in_=ar_f[c][:])
            for m in range(KD):
                nc.vector.scalar_tensor_tensor(
                    out=xt[:, m, cs], in0=art[:, m, :],
                    scalar=b2_t[:, m:m + 1],
                    in1=xt[:, m, cs].bitcast(F32),
                    op0=ALU.add, op1=ALU.add)
            layer_norm_chunk(xt, n_next, c)

        for l in range(L):
            # ------- attention + Wo + AR, chunk-pipelined -------
            oT = op.tile([128, DQ // 128, T], BF16, tag="oT")
            n2 = npool.tile([128, KD, T], BF16, tag="n")
            ar_a = []

            def add_a(c):
                art = arr.tile([128, KD, TCH], BF16, tag="arr",
                               name=f"arta{c}_{l}")
                nc.sync.dma_start(out=art[:], in_=ar_a[c][:])
                nc.vector.tensor_tensor(
                    out=xt[:, :, ts(c, TCH)],
                    in0=xt[:, :, ts(c, TCH)].bitcast(F32),
                    in1=art[:], op=ALU.add)

            def add_a_ln2_c0():
                add_a(0)
                layer_norm_chunk(xt, n2, 0)

            attn_chunk(l, 0, qT, kT, vhs, oT)
            ar_a.append(mm_ar_chunk(_f("wo", l), DQ // 128, oT, 0, False))
            attn_chunk(l, 1, qT, kT, vhs, oT, inject=add_a_ln2_c0)
            ar_a.append(mm_ar_chunk(_f("wo", l), DQ // 128, oT, 1, False))

            # ------- FFN chunk 0 first: covers the in-flight AR_a1 -------
            b1_t = bias.tile([128, FFC // 128], F32, tag="bias")
            nc.sync.dma_start(out=b1_t[:], in_=inp[_f("b1", l)][:])
            b2_t = bias.tile([128, KD], F32, tag="bias")
            nc.sync.dma_start(out=b2_t[:], in_=inp[_f("b2", l)][:])

            ar_f = []
            hT0 = ffn_w1(l, 0, n2, b1_t)
            ar_f.append(mm_ar_chunk(_f("w2", l), FFC // 128, hT0, 0, True))
            # residual + LN2 for chunk 1 (needs AR_a1, now covered)
            add_a(1)
            layer_norm_chunk(xt, n2, 1)
            hT1 = ffn_w1(l, 1, n2, b1_t)

            n_next = npool.tile([128, KD, T], BF16, tag="n")
            ar_f.append(mm_ar_chunk(_f("w2", l), FFC // 128, hT1, 1, True))
            add_f_lnnext(0, ar_f, b2_t, n_next)

            if l < L - 1:
                bqkv_t = bias.tile([128, 6], F32, tag="bias")
                nc.sync.dma_start(out=bqkv_t[:], in_=inp[_f("bqkv", l + 1)][:])
                qT = qkv.tile([128, DQ // 128, T], BF16, tag="qT")
                kT = qkv.tile([128, DQ // 128, T], BF16, tag="kT")
                vT = qkv.tile([128, DQ // 128, T], BF16, tag="vT")
                vhs = [vh.tile([128, KD, HD], BF16, tag="vh",
                               name=f"vh{l + 1}_{i}") for i in range(HPC)]
                qkv_head_chunk(l + 1, 0, n_next, qT, kT, vT, bqkv_t, vhs)
            add_f_lnnext(1, ar_f, b2_t, n_next)
            if l < L - 1:
                qkv_head_chunk(l + 1, 1, n_next, qT, kT, vT, bqkv_t, vhs)
            n_cur = n_next

        # after the loop, n_cur holds the final layer norm
        nf = n_cur

        # logits^T [VSP, T]: emb strip is the stationary operand, nf moves.
        # A t2=0-only prefix runs first: it depends only on chunk 0 of the
        # final layernorm, hiding the last AllReduce + LN tail of chunk 1.
        NPRE = 16

        def logit_block(vb, t2, ebt, use_dve):
            pt = ps_mm.tile([128, TCH], F32, tag="mm",
                            name=f"plg{vb}_{t2}")
            for k in range(KD):
                nc.tensor.matmul(pt[:], ebt[:, k, :],
                                 nf[:, k, ts(t2, TCH)],
                                 start=(k == 0), stop=(k == KD - 1))
            lo = lout.tile([128, TCH], BF16, tag="lo", name=f"lo{vb}_{t2}")
            if use_dve:
                nc.vector.tensor_scalar_mul(lo[:], pt[:], 1.0)
            else:
                nc.scalar.copy(lo[:], pt[:])
            nc.sync.dma_start(out=logits[ts(vb, 128), ts(t2, TCH)],
                              in_=lo[:])

        for vb in range(NPRE):
            ebt = embp.tile([128, KD, 128], BF16, tag="emb", name=f"ebA{vb}")
            nc.sync.dma_start(out=ebt[:], in_=inp["emb"][:, :, ts(vb, 128)])
            logit_block(vb, 0, ebt, vb % 2 == 0)
        for vb in range(VSP // 128):
            ebt = embp.tile([128, KD, 128], BF16, tag="emb", name=f"ebB{vb}")
            nc.sync.dma_start(out=ebt[:], in_=inp["emb"][:, :, ts(vb, 128)])
            for t2 in range(NTC):
                if t2 == 0 and vb < NPRE:
                    continue
                logit_block(vb, t2, ebt, vb % 2 == 0)


# ------------------------------------------------------------------
# Host side
# ------------------------------------------------------------------

def _kfold(w):
    """[in, out] -> [128, in//128, out] K-tiled layout."""
    i, o = w.shape
    return np.ascontiguousarray(
        w.reshape(i // 128, 128, o).transpose(1, 0, 2))


def _bf(w):
    return np.ascontiguousarray(w).astype(ml_dtypes.bfloat16)


def _cols(v):
    """[n] -> [128, n//128] per-partition bias columns."""
    return np.ascontiguousarray(v.reshape(-1, 128).T)


def prep_inputs(inputs):
    """Full inputs -> list of 8 per-core input maps."""
    f = lambda a: np.asarray(a, np.float32)
    tokens = np.asarray(inputs["tokens"])
    tok_emb, pos_emb = f(inputs["tok_emb"]), f(inputs["pos_emb"])
    ln1_g, ln1_b = f(inputs["ln1_g"]), f(inputs["ln1_b"])
    wq, wk = f(inputs["wq"]), f(inputs["wk"])
    wv, wo = f(inputs["wv"]), f(inputs["wo"])
    ln2_g, ln2_b = f(inputs["ln2_g"]), f(inputs["ln2_b"])
    w1, b1 = f(inputs["w1"]), f(inputs["b1"])
    w2, b2 = f(inputs["w2"]), f(inputs["b2"])
    lnf_g = f(inputs["lnf_g"])

    sc = 1.0 / np.sqrt(HD)
    x0 = tok_emb[tokens] + pos_emb[:S][None]          # [B, S, D]
    ones = np.ones((128, 1), np.float32)
    ident = np.eye(128, dtype=np.float32)
    # causal mask for transposed scores sT[k, q]: -inf-ish where k > q
    triT = np.tril(np.full((128, 128), MASK_VAL, np.float32), k=-1)

    in_maps = []
    for core in range(N_CORES):
        b = core // TP
        tpr = core % TP
        m = {
            "x0": _kfold(np.ascontiguousarray(x0[b].T)),
            "ones": ones, "ones16": _bf(ones),
            "ident": _bf(ident), "triT": _bf(triT),
            "ident2": _bf(np.vstack([np.eye(64), np.eye(64)]).astype(np.float32)),
        }
        qs = slice(tpr * DQ, (tpr + 1) * DQ)
        fs = slice(tpr * FFC, (tpr + 1) * FFC)
        for l in range(L):
            wql = wq[l][:, qs] * sc
            wkl = wk[l][:, qs]
            wvl = wv[l][:, qs]
            m[_f("wq", l)] = _bf(_kfold(ln1_g[l][:, None] * wql))
            m[_f("wk", l)] = _bf(_kfold(ln1_g[l][:, None] * wkl))
            m[_f("wv", l)] = _bf(_kfold(ln1_g[l][:, None] * wvl))
            m[_f("wo", l)] = _bf(_kfold(wo[l][qs, :]))
            m[_f("w1", l)] = _bf(_kfold(ln2_g[l][:, None] * w1[l][:, fs]))
            m[_f("w2", l)] = _bf(_kfold(w2[l][fs, :]))
            m[_f("bqkv", l)] = np.concatenate(
                [_cols(ln1_b[l] @ wql), _cols(ln1_b[l] @ wkl),
                 _cols(ln1_b[l] @ wvl)], axis=1)
            m[_f("b1", l)] = _cols(b1[l][fs] + ln2_b[l] @ w1[l][:, fs])
            m[_f("b2", l)] = _cols(b2[l])
        v0 = tpr * VS
        v1 = min(v0 + VS, V)
        epad = np.zeros((D, VSP), np.float32)
        epad[:, :v1 - v0] = (tok_emb[v0:v1] * lnf_g[None, :]).T
        m["emb"] = _bf(_kfold(epad))
        in_maps.append(m)
    return in_maps


_CACHED = {}


def _get_program():
    if "nc" not in _CACHED:
        _CACHED["nc"] = build_program()
    return _CACHED["nc"]


def run(inputs, trace=False, **kw):
    nc = _get_program()
    in_maps = prep_inputs(inputs)
    return run_bass_kernel_spmd(nc, in_maps, list(range(N_CORES)),
                                trace=trace, **kw)


def assemble(results, inputs):
    """Per-core logits -> full [B, S, V] float32."""
    lnf_b = np.asarray(inputs["lnf_b"], np.float32)
    tok_emb = np.asarray(inputs["tok_emb"], np.float32)
    out = np.empty((B, S, V), np.float32)
    for b in range(B):
        parts = []
        for tpr in range(TP):
            v0 = tpr * VS
            v1 = min(v0 + VS, V)
            parts.append(
                results[b * TP + tpr]["logits"][:v1 - v0, :]
                .astype(np.float32).T)
        out[b] = np.concatenate(parts, axis=1)
    if np.any(lnf_b):
        out += (tok_emb @ lnf_b)[None, None, :]
    return out


def kernel(**inputs):
    res = run(inputs)
    return assemble(res.results, inputs)


if __name__ == "__main__":
    print("building program...")
    build_program()
    print("build + compile OK")


# revision 10
# speedup vs baseline: 2.0144x; 1.1834x over previous
"""GPT forward pass on 8 Trainium2 NeuronCores (Bass/Tile) — v4.

Model: B=2, S=1024, D=1024, H=16 heads (hd=64), L=6 layers, V=50257,
tied embedding head.

Sharding: DP2 x TP4. Cores 0-3 compute batch element 0, cores 4-7
batch element 1. Within each group of 4: attention is head-sharded
(4 heads/core), the FFN hidden dim is column/row sharded (1024/core),
and the tied logit matrix is vocab-sharded. Two bf16 AllReduces per
layer (post-attention, post-FFN), token-chunked (2 x 512).

v4 highlights:
- bf16 everywhere except the residual stream / LN stats (f32).
- Cross-layer software pipeline: attention chunk 0 of layer l+1 only
  needs chunk 0 of its QKV, so it runs between the FFN AllReduce of
  layer l and the residual add that consumes it — every AR is covered
  by independent PE work, and the PE stays dense enough to hold its
  full clock (HAM ramps the clock down on idle gaps).
- Per-layer weights are DMA'd once into big SBUF tiles (v3 re-loaded
  every 128-col strip per token chunk: 2x the bytes, ~100 descriptor
  issues per layer).
- Scores computed transposed (sT[k,q]) so exp() output feeds the AV
  matmul directly; softmax sums via a ones-matmul; normalization
  folded into the PSUM->SBUF output pass.
- LN rsqrt = exp(-0.5*ln(var+eps)); the Act-table membership map is
  patched so Ln and Exp resolve to the combined natural_log_exp set
  (the greedy table picker otherwise alternates two sets every LN).
- AllReduce staging coalesced to one DMA per matmul group.
"""

import sys

sys.path.insert(0, "/opt/trn_rl_repo")

import contextlib

import ml_dtypes
import numpy as np

import concourse.bacc as bacc
import concourse.mybir as mybir
import concourse.tile as tile
from concourse.bass import ts
from concourse.bass_utils import run_bass_kernel_spmd
from concourse.hw_specs import get_activation_tables

F32 = mybir.dt.float32
F32R = mybir.dt.float32r
BF16 = mybir.dt.bfloat16
AF = mybir.ActivationFunctionType
ALU = mybir.AluOpType

# Model dims
B, S, D, H, L, V = 2, 1024, 1024, 16, 6, 50257
HD = D // H           # 64
DFF = 4 * D           # 4096
N_CORES = 8
TP = 4                # tensor-parallel group size
HPC = H // TP         # heads per core = 4
DQ = HPC * HD         # per-core qkv width = 256
FFC = DFF // TP       # per-core ffn hidden = 1024
KD = D // 128         # 8 feature tiles
T = S                 # tokens per core (one batch element)
TCH = 512             # token chunk for AR pipelining
NTC = T // TCH        # 2
VS = 12565            # vocab rows per core (last core: 12562)
VSP = 12800           # padded vocab rows per core
MASK_VAL = -60.0
NPRE = 16             # vocab blocks in the chunk-0-only logit prefix

REPLICA_GROUPS = [[0, 1, 2, 3], [4, 5, 6, 7]]


def _f(name, l=None):
    return name if l is None else f"{name}{l}"


def _patch_act_tables(arch):
    """Make the greedy act-table picker resolve Ln and Exp to the
    combined natural_log_exp set instead of two separate sets (which
    costs two table loads around every layer norm)."""
    tabs = get_activation_tables(arch)   # cached dict: mutate in place
    if "natural_log_exp_and_others" not in tabs:
        return
    tabs["natural_log"].discard(AF.Ln)
    tabs["exp_and_others"].discard(AF.Exp)


def build_program():
    """Build the SPMD bass program (same instruction stream on all cores)."""
    nc = bacc.Bacc("TRN2", target_bir_lowering=False, debug=False,
                   enable_asserts=True, num_devices=N_CORES)
    _patch_act_tables(nc.m.arch)

    inp = {}

    def dram_in(name, shape, dtype=BF16):
        inp[name] = nc.dram_tensor(name, shape, dtype, kind="ExternalInput").ap()
        return inp[name]

    dram_in("x0", [128, KD, T])
    dram_in("ones", [128, 1], F32R)
    dram_in("ones16", [128, 1])
    dram_in("ident", [128, 128])
    dram_in("triT", [128, 128])
    dram_in("ident2", [128, 64])
    for l in range(L):
        dram_in(_f("wqkv", l), [128, KD, 3 * DQ])
        dram_in(_f("wo", l), [128, DQ // 128, D])
        dram_in(_f("w1", l), [128, KD, FFC])
        dram_in(_f("w2", l), [128, FFC // 128, D])
        dram_in(_f("bqkv", l), [128, 6], F32)
        dram_in(_f("b1", l), [128, FFC // 128], F32)
        dram_in(_f("b2", l), [128, KD], F32)
    dram_in("emb", [128, KD, VSP])
    logits = nc.dram_tensor("logits", [VSP, T], BF16, kind="ExternalOutput").ap()

    with tile.TileContext(nc) as tc:
        _body(tc, inp, logits)
    nc.compile()
    return nc


def _body(tc, inp, logits):
    nc = tc.nc
    ctx = contextlib.ExitStack()
    with ctx:
        # --- SBUF pools ---
        singles = ctx.enter_context(tc.tile_pool(name="singles", bufs=1))
        xp = ctx.enter_context(tc.tile_pool(name="xp", bufs=1))        # x f32
        npool = ctx.enter_context(tc.tile_pool(name="npool", bufs=1))
        tmp = ctx.enter_context(tc.tile_pool(name="tmp", bufs=2))
        qkv = ctx.enter_context(tc.tile_pool(name="qkv", bufs=1))
        vh = ctx.enter_context(tc.tile_pool(name="vh", bufs=4))
        ep = ctx.enter_context(tc.tile_pool(name="ep", bufs=3))
        op = ctx.enter_context(tc.tile_pool(name="op", bufs=1))
        hp = ctx.enter_context(tc.tile_pool(name="hp", bufs=1))
        arr = ctx.enter_context(tc.tile_pool(name="arr", bufs=1))
        wts = ctx.enter_context(tc.tile_pool(name="wts", bufs=1))
        embp = ctx.enter_context(tc.tile_pool(name="embp", bufs=4))
        stat = ctx.enter_context(tc.tile_pool(name="stat", bufs=2))
        bcp = ctx.enter_context(tc.tile_pool(name="bcp", bufs=1))
        lout = ctx.enter_context(tc.tile_pool(name="lout", bufs=2))
        bias = ctx.enter_context(tc.tile_pool(name="bias", bufs=3))
        stg = ctx.enter_context(tc.tile_pool(name="stg", bufs=2))
        # --- PSUM pools (8 banks total) ---
        ps_mm = ctx.enter_context(tc.tile_pool(name="ps_mm", bufs=2, space="PSUM"))
        ps_st = ctx.enter_context(tc.tile_pool(name="ps_st", bufs=1, space="PSUM"))
        ps_sc = ctx.enter_context(tc.tile_pool(name="ps_sc", bufs=2, space="PSUM"))
        ps_tr = ctx.enter_context(tc.tile_pool(name="ps_tr", bufs=1, space="PSUM"))
        ps_o = ctx.enter_context(tc.tile_pool(name="ps_o", bufs=1, space="PSUM"))
        ps_rs = ctx.enter_context(tc.tile_pool(name="ps_rs", bufs=1, space="PSUM"))
        # --- DRAM (collective bounce) ---
        dram = ctx.enter_context(tc.tile_pool(name="dram", bufs=4, space="DRAM"))

        # --- constants / persistent ---
        ones_t = singles.tile([128, 1], F32R)
        nc.sync.dma_start(out=ones_t[:], in_=inp["ones"][:])
        ones16_t = singles.tile([128, 1], BF16)
        nc.sync.dma_start(out=ones16_t[:], in_=inp["ones16"][:])
        ident_t = singles.tile([128, 128], BF16)
        nc.sync.dma_start(out=ident_t[:], in_=inp["ident"][:])
        ident2_t = singles.tile([128, 64], BF16)
        nc.sync.dma_start(out=ident2_t[:], in_=inp["ident2"][:])
        triT_t = singles.tile([128, 128], BF16)
        nc.sync.dma_start(out=triT_t[:], in_=inp["triT"][:])
        eps_t = singles.tile([1, 1], F32)
        nc.vector.memset(eps_t[:], 1e-5)
        warm_t = singles.tile([1, 1], F32)
        nc.vector.memset(warm_t[:], 1.0)
        warm_o = singles.tile([1, 1], F32)

        def act_warm(func):
            """Dummy activation that pre-loads an Act table set while
            the Act engine is idle, off the critical path."""
            nc.scalar.activation(warm_o[:], warm_t[:], func)

        def load_weights(l):
            """One big DMA per weight tensor for layer l."""
            wqkv_t = wts.tile([128, KD, 3 * DQ], BF16, tag="wqkv",
                              name=f"wqkv_t{l}", bufs=1)
            nc.sync.dma_start(out=wqkv_t[:], in_=inp[_f("wqkv", l)][:])
            wo_t = wts.tile([128, DQ // 128, D], BF16, tag="wo",
                            name=f"wo_t{l}", bufs=2)
            nc.sync.dma_start(out=wo_t[:], in_=inp[_f("wo", l)][:])
            return {"wqkv": wqkv_t, "wo": wo_t}

        def load_w1(l, w):
            w["w1"] = wts.tile([128, KD, FFC], BF16, tag="w1",
                               name=f"w1_t{l}", bufs=1)
            nc.sync.dma_start(out=w["w1"][:], in_=inp[_f("w1", l)][:])

        def load_w2(l, w):
            w["w2"] = wts.tile([128, FFC // 128, D], BF16, tag="w2",
                               name=f"w2_t{l}", bufs=1)
            nc.sync.dma_start(out=w["w2"][:], in_=inp[_f("w2", l)][:])

        xt = xp.tile([128, KD, T], F32R, tag="x")

        def layer_norm_chunk(src, dst, c):
            """dst[:,:,c] = (src - mean) * rsqrt(var + eps); dst bf16."""
            cs = ts(c, TCH)
            s1 = ps_st.tile([1, TCH], F32, tag="st")
            s2 = ps_st.tile([1, TCH], F32, tag="st")
            for k in range(KD):
                nc.tensor.matmul(s1[:], ones_t[:], src[:, k, cs],
                                 start=(k == 0), stop=(k == KD - 1))
            for k in range(KD):
                sq = tmp.tile([128, TCH], F32R, tag="tmp")
                nc.vector.tensor_tensor(
                    out=sq[:], in0=src[:, k, cs].bitcast(F32),
                    in1=src[:, k, cs].bitcast(F32), op=ALU.mult)
                nc.tensor.matmul(s2[:], ones_t[:], sq[:],
                                 start=(k == 0), stop=(k == KD - 1))
            m = stat.tile([1, TCH], F32, tag="sa")
            nc.vector.tensor_scalar_mul(m[:], s1[:], 1.0 / D)
            msq = stat.tile([1, TCH], F32, tag="sb")
            nc.vector.tensor_tensor(out=msq[:], in0=m[:], in1=m[:],
                                    op=ALU.mult)
            var = stat.tile([1, TCH], F32, tag="sb")
            nc.vector.scalar_tensor_tensor(
                out=var[:], in0=s2[:], scalar=1.0 / D, in1=msq[:],
                op0=ALU.mult, op1=ALU.subtract)
            # rsqrt(var+eps) = exp(-0.5*ln(var+eps)); Ln/Exp share the
            # attention Exp's table set — no Act table switch.
            lnv = stat.tile([1, TCH], F32, tag="sb")
            nc.scalar.activation(lnv[:], var[:], AF.Ln, bias=eps_t[:])
            rs = stat.tile([1, TCH], F32, tag="sb")
            nc.scalar.activation(rs[:], lnv[:], AF.Exp, scale=-0.5)
            a = stat.tile([1, TCH], F32, tag="sa")
            nc.vector.scalar_tensor_tensor(
                out=a[:], in0=m[:], scalar=-1.0, in1=rs[:],
                op0=ALU.mult, op1=ALU.mult)
            rB = bcp.tile([128, TCH], F32, tag="rB")
            nc.gpsimd.partition_broadcast(rB[:], rs[:])
            aB = bcp.tile([128, TCH], F32, tag="aB")
            nc.gpsimd.partition_broadcast(aB[:], a[:])
            for k in range(KD):
                t2 = tmp.tile([128, TCH], F32, tag="tmp")
                nc.vector.tensor_tensor(
                    out=t2[:], in0=src[:, k, cs].bitcast(F32),
                    in1=rB[:], op=ALU.mult)
                nc.vector.tensor_tensor(
                    out=dst[:, k, cs], in0=t2[:], in1=aB[:], op=ALU.add)

        def proj_chunk(dst, w_t, col0, n_src, mchunks, bias_t, bcol0, c):
            """dst[:, m, c-chunk] (bf16) = W^T @ n_src[c-chunk] + bias,
            with W = w_t columns [col0, col0+128*mchunks)."""
            cs = ts(c, TCH)
            for m in range(mchunks):
                pt = ps_mm.tile([128, TCH], F32, tag="mm")
                for k in range(KD):
                    nc.tensor.matmul(pt[:],
                                     w_t[:, k, col0 + 128 * m:col0 + 128 * (m + 1)],
                                     n_src[:, k, cs],
                                     start=(k == 0), stop=(k == KD - 1))
                nc.scalar.activation(
                    dst[:, m, cs], pt[:], AF.Identity,
                    bias=bias_t[:, bcol0 + m:bcol0 + m + 1])

        def qkv_head_chunk(l, c, n_src, qT, kT, vT, bqkv_t, vhs, wqkv_t):
            """QKV projections for token chunk c, plus the v-transposes
            whose key blocks live in chunk c."""
            nm = DQ // 128
            proj_chunk(qT, wqkv_t, 0, n_src, nm, bqkv_t, 0, c)
            proj_chunk(kT, wqkv_t, DQ, n_src, nm, bqkv_t, 2, c)
            proj_chunk(vT, wqkv_t, 2 * DQ, n_src, nm, bqkv_t, 4, c)
            for h in range(HPC):
                pp = 64 * (h % 2)
                mh = h // 2
                for j in range(4 * c, 4 * c + 4):
                    tp_ = ps_tr.tile([128, 128], BF16, tag="tr")
                    nc.tensor.matmul(tp_[:, 0:HD],
                                     vT[pp:pp + 64, mh, ts(j, 128)],
                                     ident2_t[pp:pp + 64, :],
                                     is_transpose=True,
                                     start=True, stop=True)
                    nc.scalar.copy(vhs[h][:, j, :], tp_[:, 0:HD])

        def attn_chunk(l, c, qT, kT, vhs, oT, inject=None, inject_at=3):
            """Attention for token chunk c with transposed scores.

            sT[k, q] comes straight from a stationary-k / moving-q
            matmul, so exp(sT) lands in SBUF in the layout the AV
            matmul consumes. Softmax sums via a ones-matmul accumulated
            across key blocks; normalization folded into the single
            PSUM->SBUF output pass (GpSimd row broadcast + DVE mult).
            """
            nkj = 4 * (c + 1)
            for h in range(HPC):
                if h == inject_at and inject is not None:
                    inject()
                pp = 64 * (h % 2)
                mh = h // 2
                po = ps_o.tile([64, TCH], F32, tag="o")
                rsp = ps_rs.tile([1, TCH], F32, tag="rs")

                def score_exp(kj):
                    lo = max(0, 128 * kj - TCH * c)
                    diag = 128 * kj >= TCH * c
                    spt = ps_sc.tile([128, TCH], F32, tag="sc")
                    nc.tensor.matmul(
                        spt[:, lo:TCH],
                        kT[pp:pp + 64, mh, ts(kj, 128)],
                        qT[pp:pp + 64, mh, TCH * c + lo:TCH * (c + 1)],
                        start=True, stop=not diag)
                    if diag:
                        nc.tensor.matmul(spt[:, lo:lo + 128], ident_t[:],
                                         triT_t[:], start=False, stop=True)
                    eT = ep.tile([128, TCH], BF16, tag="e")
                    nc.scalar.activation(eT[:, lo:TCH], spt[:, lo:TCH],
                                         AF.Exp)
                    return eT

                def sum_av(kj, eT):
                    lo = max(0, 128 * kj - TCH * c)
                    nc.tensor.matmul(rsp[:, lo:TCH], ones16_t[:],
                                     eT[:, lo:TCH],
                                     start=(kj == 0), stop=(kj == nkj - 1))
                    nc.tensor.matmul(po[:, lo:TCH], vhs[h][:, kj, :],
                                     eT[:, lo:TCH],
                                     start=(kj == 0), stop=(kj == nkj - 1))

                prev = score_exp(0)
                for kj in range(1, nkj):
                    cur = score_exp(kj)
                    sum_av(kj - 1, prev)
                    prev = cur
                sum_av(nkj - 1, prev)

                rinv = stat.tile([1, TCH], F32, tag="ri")
                nc.vector.reciprocal_approx_fast(out=rinv[:], in_=rsp[:])
                rB = bcp.tile([128, TCH], F32, tag="rA")
                nc.gpsimd.partition_broadcast(rB[:], rinv[:])
                nc.vector.tensor_tensor(
                    out=oT[pp:pp + 64, mh, ts(c, TCH)], in0=po[:],
                    in1=rB[0:64, :], op=ALU.mult)

        def mm_ar_chunk(w_t, kchunks, src_tile, c, src_is_chunk):
            """out-partial = W^T @ src for chunk c -> bf16 AllReduce.
            Staged through one SBUF tile and a single DMA."""
            ar_in = dram.tile([128, KD, TCH], BF16, tag="dr")
            ar_out = dram.tile([128, KD, TCH], BF16, tag="dr")
            st_ = stg.tile([128, KD, TCH], BF16, tag="stg")
            for m in range(KD):
                pt = ps_mm.tile([128, TCH], F32, tag="mm")
                for k in range(kchunks):
                    s = (src_tile[:, k, :] if src_is_chunk
                         else src_tile[:, k, ts(c, TCH)])
                    nc.tensor.matmul(pt[:], w_t[:, k, ts(m, 128)], s,
                                     start=(k == 0), stop=(k == kchunks - 1))
                nc.scalar.copy(st_[:, m, :], pt[:])
            nc.sync.dma_start(out=ar_in[:], in_=st_[:])
            nc.gpsimd.collective_compute(
                "AllReduce", ALU.add, replica_groups=REPLICA_GROUPS,
                ins=[ar_in.opt()], outs=[ar_out.opt()])
            return ar_out

        def ffn_w1(l, c, n2, b1_t, w1_t):
            cs = ts(c, TCH)
            hT = hp.tile([128, FFC // 128, TCH], BF16, tag="h",
                         name=f"hT{l}_{c}")
            for m in range(FFC // 128):
                pt = ps_mm.tile([128, TCH], F32, tag="mm", name=f"p1_{l}_{c}_{m}")
                for k in range(KD):
                    nc.tensor.matmul(pt[:], w1_t[:, k, ts(m, 128)],
                                     n2[:, k, cs],
                                     start=(k == 0), stop=(k == KD - 1))
                nc.scalar.activation(hT[:, m, :], pt[:], AF.Gelu,
                                     bias=b1_t[:, m:m + 1])
            # pre-load the Ln/Exp table set while Act is idle
            act_warm(AF.Ln)
            return hT

        def add_f_lnnext(c, ar_f, b2_t, n_next):
            cs = ts(c, TCH)
            art = arr.tile([128, KD, TCH], BF16, tag="arr", name=f"artf{c}")
            nc.sync.dma_start(out=art[:], in_=ar_f[c][:])
            for m in range(KD):
                nc.vector.scalar_tensor_tensor(
                    out=xt[:, m, cs], in0=art[:, m, :],
                    scalar=b2_t[:, m:m + 1],
                    in1=xt[:, m, cs].bitcast(F32),
                    op0=ALU.add, op1=ALU.add)
            layer_norm_chunk(xt, n_next, c)

        def logit_block(vb, t2, ebt, use_dve):
            pt = ps_mm.tile([128, TCH], F32, tag="mm",
                            name=f"plg{vb}_{t2}")
            for k in range(KD):
                nc.tensor.matmul(pt[:], ebt[:, k, :],
                                 n_cur[:, k, ts(t2, TCH)],
                                 start=(k == 0), stop=(k == KD - 1))
            lo = lout.tile([128, TCH], BF16, tag="lo", name=f"lo{vb}_{t2}")
            if use_dve:
                nc.vector.tensor_scalar_mul(lo[:], pt[:], 1.0)
            else:
                nc.scalar.copy(lo[:], pt[:])
            nc.sync.dma_start(out=logits[ts(vb, 128), ts(t2, TCH)],
                              in_=lo[:])

        # ---------------- prologue ----------------
        w_cur = load_weights(0)
        load_w1(0, w_cur)
        load_w2(0, w_cur)
        n_cur = npool.tile([128, KD, T], BF16, tag="n", name="n_l0")
        bqkv_t = bias.tile([128, 6], F32, tag="bias")
        nc.sync.dma_start(out=bqkv_t[:], in_=inp[_f("bqkv", 0)][:])
        qT = qkv.tile([128, DQ // 128, T], BF16, tag="qT")
        kT = qkv.tile([128, DQ // 128, T], BF16, tag="kT")
        vT = qkv.tile([128, DQ // 128, T], BF16, tag="vT")
        vhs = [vh.tile([128, KD, HD], BF16, tag="vh", name=f"vh{i}")
               for i in range(HPC)]
        for c in range(NTC):
            # x0 arrives bf16 in chunks; upcast into the f32 residual
            x0s = arr.tile([128, KD, TCH], BF16, tag="arr", name=f"x0s{c}")
            nc.sync.dma_start(out=x0s[:], in_=inp["x0"][:, :, ts(c, TCH)])
            nc.scalar.copy(xt[:, :, ts(c, TCH)], x0s[:])
            layer_norm_chunk(xt, n_cur, c)
            qkv_head_chunk(0, c, n_cur, qT, kT, vT, bqkv_t, vhs,
                           w_cur["wqkv"])

        oT = op.tile([128, DQ // 128, T], BF16, tag="oT", name="oT_l0")
        n2 = npool.tile([128, KD, T], BF16, tag="n", name="n2_l0")
        attn_chunk(0, 0, qT, kT, vhs, oT)
        ar_a = [mm_ar_chunk(w_cur["wo"], DQ // 128, oT, 0, False)]

        # ---------------- main loop: layers, software-pipelined ----------------
        for l in range(L):
            def add_a(c):
                art = arr.tile([128, KD, TCH], BF16, tag="arr",
                               name=f"arta{c}_{l}")
                nc.sync.dma_start(out=art[:], in_=ar_a[c][:])
                nc.vector.tensor_tensor(
                    out=xt[:, :, ts(c, TCH)],
                    in0=xt[:, :, ts(c, TCH)].bitcast(F32),
                    in1=art[:], op=ALU.add)

            def add_a_ln2_c0():
                add_a(0)
                layer_norm_chunk(xt, n2, 0)

            # 1-2: attention chunk 1 + its Wo partial/AR
            attn_chunk(l, 1, qT, kT, vhs, oT, inject=add_a_ln2_c0)
            act_warm(AF.Gelu)
            ar_a.append(mm_ar_chunk(w_cur["wo"], DQ // 128, oT, 1, False))
            if l < L - 1:
                w_next = load_weights(l + 1)
                bqkv_t = bias.tile([128, 6], F32, tag="bias")
                nc.sync.dma_start(out=bqkv_t[:], in_=inp[_f("bqkv", l + 1)][:])

            # 3-4: FFN chunk 0 (covers AR_a1)
            b1_t = bias.tile([128, FFC // 128], F32, tag="bias")
            nc.sync.dma_start(out=b1_t[:], in_=inp[_f("b1", l)][:])
            b2_t = bias.tile([128, KD], F32, tag="bias")
            nc.sync.dma_start(out=b2_t[:], in_=inp[_f("b2", l)][:])
            ar_f = []
            hT0 = ffn_w1(l, 0, n2, b1_t, w_cur["w1"])
            ar_f.append(mm_ar_chunk(w_cur["w2"], FFC // 128, hT0, 0, True))

            # 5-7: residual+LN2 chunk 1, FFN chunk 1
            add_a(1)
            layer_norm_chunk(xt, n2, 1)
            act_warm(AF.Gelu)
            hT1 = ffn_w1(l, 1, n2, b1_t, w_cur["w1"])
            if l < L - 1:
                load_w1(l + 1, w_next)
            ar_f.append(mm_ar_chunk(w_cur["w2"], FFC // 128, hT1, 1, True))
            if l < L - 1:
                load_w2(l + 1, w_next)

            # 8: residual + next-layer LN1, chunk 0 (needs AR_f0)
            n_next = npool.tile([128, KD, T], BF16, tag="n", name=f"nn{l}")
            add_f_lnnext(0, ar_f, b2_t, n_next)

            if l < L - 1:
                # 9-11: next layer's QKV chunk 0 + attention chunk 0 +
                # Wo AR chunk 0 — all only need chunk 0 of n_next, and
                # together they cover the in-flight AR_f1.
                qT = qkv.tile([128, DQ // 128, T], BF16, tag="qT",
                              name=f"qT{l + 1}")
                kT = qkv.tile([128, DQ // 128, T], BF16, tag="kT",
                              name=f"kT{l + 1}")
                vT = qkv.tile([128, DQ // 128, T], BF16, tag="vT",
                              name=f"vT{l + 1}")
                vhs = [vh.tile([128, KD, HD], BF16, tag="vh",
                               name=f"vh{l + 1}_{i}") for i in range(HPC)]
                qkv_head_chunk(l + 1, 0, n_next, qT, kT, vT, bqkv_t, vhs,
                               w_next["wqkv"])
                oT = op.tile([128, DQ // 128, T], BF16, tag="oT",
                             name=f"oT{l + 1}")
                n2 = npool.tile([128, KD, T], BF16, tag="n", name=f"n2_{l + 1}")
                attn_chunk(l + 1, 0, qT, kT, vhs, oT)
                ar_a = [mm_ar_chunk(w_next["wo"], DQ // 128, oT, 0, False)]
                # 12-13
                add_f_lnnext(1, ar_f, b2_t, n_next)
                qkv_head_chunk(l + 1, 1, n_next, qT, kT, vT, bqkv_t, vhs,
                               w_next["wqkv"])
                w_cur = w_next
            else:
                # last layer: n_next is the final layer norm. Run the
                # chunk-0-only logit prefix before the chunk-1 residual
                # so it covers the in-flight AR_f1.
                n_cur = n_next
                for vb in range(NPRE):
                    ebt = embp.tile([128, KD, 128], BF16, tag="emb",
                                    name=f"ebA{vb}")
                    nc.sync.dma_start(out=ebt[:],
                                      in_=inp["emb"][:, :, ts(vb, 128)])
                    logit_block(vb, 0, ebt, vb % 2 == 0)
                add_f_lnnext(1, ar_f, b2_t, n_next)

        # ---------------- logits ----------------
        for vb in range(VSP // 128):
            ebt = embp.tile([128, KD, 128], BF16, tag="emb", name=f"ebB{vb}")
            nc.sync.dma_start(out=ebt[:], in_=inp["emb"][:, :, ts(vb, 128)])
            for t2 in range(NTC):
                if t2 == 0 and vb < NPRE:
                    continue
                logit_block(vb, t2, ebt, vb % 2 == 0)


# ------------------------------------------------------------------
# Host side
# ------------------------------------------------------------------

def _kfold(w):
    """[in, out] -> [128, in//128, out] K-tiled layout."""
    i, o = w.shape
    return np.ascontiguousarray(
        w.reshape(i // 128, 128, o).transpose(1, 0, 2))


def _bf(w):
    return np.ascontiguousarray(w).astype(ml_dtypes.bfloat16)


def _cols(v):
    """[n] -> [128, n//128] per-partition bias columns."""
    return np.ascontiguousarray(v.reshape(-1, 128).T)


def prep_inputs(inputs):
    """Full inputs -> list of 8 per-core input maps."""
    f = lambda a: np.asarray(a, np.float32)
    tokens = np.asarray(inputs["tokens"])
    tok_emb, pos_emb = f(inputs["tok_emb"]), f(inputs["pos_emb"])
    ln1_g, ln1_b = f(inputs["ln1_g"]), f(inputs["ln1_b"])
    wq, wk = f(inputs["wq"]), f(inputs["wk"])
    wv, wo = f(inputs["wv"]), f(inputs["wo"])
    ln2_g, ln2_b = f(inputs["ln2_g"]), f(inputs["ln2_b"])
    w1, b1 = f(inputs["w1"]), f(inputs["b1"])
    w2, b2 = f(inputs["w2"]), f(inputs["b2"])
    lnf_g = f(inputs["lnf_g"])

    sc = 1.0 / np.sqrt(HD)
    x0 = tok_emb[tokens] + pos_emb[:S][None]          # [B, S, D]
    ones = np.ones((128, 1), np.float32)
    ident = np.eye(128, dtype=np.float32)
    # causal mask for transposed scores sT[k, q]: -inf-ish where k > q
    triT = np.tril(np.full((128, 128), MASK_VAL, np.float32), k=-1)

    in_maps = []
    for core in range(N_CORES):
        b = core // TP
        tpr = core % TP
        m = {
            "x0": _bf(_kfold(np.ascontiguousarray(x0[b].T))),
            "ones": ones, "ones16": _bf(ones),
            "ident": _bf(ident), "triT": _bf(triT),
            "ident2": _bf(np.vstack([np.eye(64), np.eye(64)]).astype(np.float32)),
        }
        qs = slice(tpr * DQ, (tpr + 1) * DQ)
        fs = slice(tpr * FFC, (tpr + 1) * FFC)
        for l in range(L):
            wql = wq[l][:, qs] * sc
            wkl = wk[l][:, qs]
            wvl = wv[l][:, qs]
            m[_f("wqkv", l)] = _bf(np.concatenate(
                [_kfold(ln1_g[l][:, None] * wql),
                 _kfold(ln1_g[l][:, None] * wkl),
                 _kfold(ln1_g[l][:, None] * wvl)], axis=2))
            m[_f("wo", l)] = _bf(_kfold(wo[l][qs, :]))
            m[_f("w1", l)] = _bf(_kfold(ln2_g[l][:, None] * w1[l][:, fs]))
            m[_f("w2", l)] = _bf(_kfold(w2[l][fs, :]))
            m[_f("bqkv", l)] = np.concatenate(
                [_cols(ln1_b[l] @ wql), _cols(ln1_b[l] @ wkl),
                 _cols(ln1_b[l] @ wvl)], axis=1)
            m[_f("b1", l)] = _cols(b1[l][fs] + ln2_b[l] @ w1[l][:, fs])
            m[_f("b2", l)] = _cols(b2[l])
        v0 = tpr * VS
        v1 = min(v0 + VS, V)
        epad = np.zeros((D, VSP), np.float32)
        epad[:, :v1 - v0] = (tok_emb[v0:v1] * lnf_g[None, :]).T
        m["emb"] = _bf(_kfold(epad))
        in_maps.append(m)
    return in_maps


_CACHED = {}


def _get_program():
    if "nc" not in _CACHED:
        _CACHED["nc"] = build_program()
    return _CACHED["nc"]


def run(inputs, trace=False, **kw):
    nc = _get_program()
    in_maps = prep_inputs(inputs)
    return run_bass_kernel_spmd(nc, in_maps, list(range(N_CORES)),
                                trace=trace, **kw)


def assemble(results, inputs):
    """Per-core logits -> full [B, S, V] float32."""
    lnf_b = np.asarray(inputs["lnf_b"], np.float32)
    tok_emb = np.asarray(inputs["tok_emb"], np.float32)
    out = np.empty((B, S, V), np.float32)
    for b in range(B):
        parts = []
        for tpr in range(TP):
            v0 = tpr * VS
            v1 = min(v0 + VS, V)
            parts.append(
                results[b * TP + tpr]["logits"][:v1 - v0, :]
                .astype(np.float32).T)
        out[b] = np.concatenate(parts, axis=1)
    if np.any(lnf_b):
        out += (tok_emb @ lnf_b)[None, None, :]
    return out


def kernel(**inputs):
    res = run(inputs)
    return assemble(res.results, inputs)


if __name__ == "__main__":
    print("building program...")
    build_program()
    print("build + compile OK")
